# revision 1
# baseline (speedup 1.0000x reference)
"""Multi-head causal self-attention (B=2, S=2048, D=1024, H=16) on 8 TRN2 cores.

Sharding: head-parallel. Core c owns head-group c = heads {2c, 2c+1}
(= 128 of the 1024 qkv dims, both batches).

Per core:
  stage B: Q^T/K^T/V^T = (x @ W{q,k,v}[:, c-slice] + b)^T          [128, 4096]
  stage C: V^T -> V_aug [tok, 65] tiles (col 64 = ones, for the l-row trick)
  stage D: per (batch, head) pair: scores^T = K^T.T-tiles @ Q^T (PE),
           causal mask (DVE), exp (ACT, scale=1/8), ctx^T accum (PE) with
           the ones column producing l = sum(exp) in row 64.
  stage E: r = 1/l (DVE reciprocal_approx), broadcast via PE outer product,
           normalize ctx^T (DVE). AllGather ctx^T across the 8 cores.
  stage F: out^T[:, c-cols] = Wo[:, c-slice].T-tiles @ gathered ctx^T + bo.

Host: passes x pre-transposed, weight column slices; transposes y^T back.
"""

import sys

for p in ("/opt/trn_rl_repo", "/root/.axon_site/_ro/trn_rl_repo"):
    if p not in sys.path:
        sys.path.insert(0, p)

import numpy as np

import bass_rust
import concourse.bass as bass
import concourse.mybir as mybir
from concourse.bass_utils import run_bass_kernel_spmd
from concourse.masks import make_identity
from concourse.tile import TileContext

B, S, D = 2, 2048, 1024
H, DH = 16, 64
T = B * S              # 4096 tokens
NC = 8                 # cores
HG = D // NC           # 128 qkv dims per core (2 heads)
KT_D = D // 128        # 8 contraction tiles over d_model
INV_SCALE = 1.0 / float(np.sqrt(DH))  # 1/8
NEG = -1.0e9
F32 = mybir.dt.float32
F32R = mybir.dt.float32r
BF16 = mybir.dt.bfloat16
PO_DT = F32R  # BF16 would save ~33us but costs 8x accuracy


def _r(ap):
    return ap.bitcast(F32R)


def _split_waits(nc, max_waits=1):
    """This walrus build accepts one sync-wait per instruction; Tile sometimes
    emits more. Split extras into preceding NoOps on the same engine."""
    n = 0
    for f in nc.m.functions:
        for bb in f.blocks:
            out = []
            for inst in bb.instructions:
                si = getattr(inst, "sync_info", None)
                if si is not None and si.on_wait and len(si.on_wait) > max_waits:
                    waits = list(si.on_wait)
                    head, rest = waits[:-max_waits], waits[-max_waits:]
                    k = 0
                    while head:
                        chunk, head = head[:max_waits], head[max_waits:]
                        out.append(mybir.InstNoOp(
                            name=f"{inst.name}-wsplit-{k}", ins=[], outs=[],
                            engine=inst.engine,
                            sync_info=bass_rust.SyncInfo(on_wait=chunk, on_update=[]),
                        ))
                        k += 1
                    si.on_wait = rest
                    n += 1
                out.append(inst)
            bb.instructions = out
    return n


def build_module(repeat=1, stages="BCDEF", do_collective=True):
    nc = bass.Bass()

    xT = nc.dram_tensor("xT", [D, T], F32R, kind="ExternalInput")
    wq = nc.dram_tensor("wq", [D, HG], F32R, kind="ExternalInput")
    wk = nc.dram_tensor("wk", [D, HG], F32R, kind="ExternalInput")
    wv = nc.dram_tensor("wv", [D, HG], F32R, kind="ExternalInput")
    wo = nc.dram_tensor("wo", [HG, D], F32R, kind="ExternalInput")
    bq = nc.dram_tensor("bq", [HG, 1], F32, kind="ExternalInput")
    bk = nc.dram_tensor("bk", [HG, 1], F32, kind="ExternalInput")
    bv = nc.dram_tensor("bv", [HG, 1], F32, kind="ExternalInput")
    bo = nc.dram_tensor("bo", [HG, 1], F32, kind="ExternalInput")
    yT = nc.dram_tensor("yT", [HG, T], F32, kind="ExternalOutput")

    # per-batch partial out^T and reduce-scatter buffers: batch 0's RS
    # overlaps batch 1's attention; only batch 1's RS is an exposed tail
    po_b = [nc.dram_tensor(f"po{i}", [D, S], PO_DT) for i in range(B)]
    rs_b = [nc.dram_tensor(f"rs{i}", [HG, S], PO_DT) for i in range(B)]

    with TileContext(nc) as tc:
        with tc.tile_pool(name="persist", bufs=1) as pp:
            # weights as [128, kt, 128]
            w_sb = {}
            for name, dram in (("wq", wq), ("wk", wk), ("wv", wv)):
                t = pp.tile([128, KT_D, HG], F32R, name=f"{name}_sb", tag=f"{name}_sb")
                nc.sync.dma_start(out=t[:], in_=dram[:].rearrange("(kt p) n -> p kt n", p=128))
                w_sb[name] = t
            # wo: [HG rows of Wo, D out dims] -> [128, ot, 128]
            wo_sb = pp.tile([128, KT_D, 128], F32R, name="wo_sb", tag="wo_sb")
            nc.sync.dma_start(out=wo_sb[:], in_=wo[:].rearrange("p (ot n) -> p ot n", n=128))
            b_sb = {}
            for name, dram in (("bq", bq), ("bk", bk), ("bv", bv), ("bo", bo)):
                t = pp.tile([HG, 1], F32, name=f"{name}_sb", tag=f"{name}_sb")
                nc.sync.dma_start(out=t[:], in_=dram[:])
                b_sb[name] = t

            # identity built in f32 (gpsimd memset can't write f32r),
            # then DVE-copied (rounds) into the f32r tile matmul needs
            ident_f = pp.tile([128, 128], F32, name="ident_f", tag="ident_f")
            make_identity(nc, ident_f[:])
            ident = pp.tile([128, 128], F32R, name="ident", tag="ident")
            nc.vector.tensor_copy(ident[:], ident_f[:])
            # additive causal mask for a diagonal 128x128 tile in scores^T
            # layout: tri[r, c] = 0 where r <= c (k <= q), else -1e9
            tri01 = pp.tile([128, 128], F32, name="tri01", tag="tri01")
            nc.gpsimd.memset(tri01[:], 1.0)
            # keep 1 where c - r >= 0 (k <= q); 0 strictly below the diagonal
            nc.gpsimd.affine_select(
                out=tri01[:], in_=tri01[:],
                compare_op=mybir.AluOpType.is_ge, fill=0.0,
                base=0, pattern=[[1, 128]], channel_multiplier=-1,
            )
            # ones on partition row 64 (same base partition as the l row)
            ones_sb = pp.tile([65, 128], F32, name="ones_sb", tag="ones_sb")
            nc.vector.memset(ones_sb[:], 1.0)
            ones128 = pp.tile([128, 64], F32, name="ones128", tag="ones128")
            nc.vector.memset(ones128[:], 1.0)
            ones_r = pp.tile([65, 128], F32R, name="ones_r", tag="ones_r")
            nc.vector.tensor_copy(ones_r[:], ones_sb[:])

            # per-batch Q^T/K^T/V^T so batch 1's projection overlaps batch 0's
            # attention
            qkvT = {}
            for name in ("qT", "kT", "vT"):
                qkvT[name] = [pp.tile([128, S], F32R, name=f"{name}{b}", tag=f"{name}{b}")
                              for b in range(B)]

            vaug = pp.tile([128, B * 2, S // 128, DH + 1], F32R, name="vaug", tag="vaug")
            nc.vector.tensor_copy(vaug[:, :, :, DH:DH + 1], ones128[:, :])
            # [65 used partitions, pair, q]; row 64 = l
            ctxu = pp.tile([128, B * 2, S], F32, name="ctxu", tag="ctxu")
            ctxn = pp.tile([128, T], F32R, name="ctxn", tag="ctxn")

            for _rep in range(repeat):
                if "B" not in stages:
                    break
                # ---------------- stage B: QKV projections (both batches) ----------------
                with (
                    tc.tile_pool(name="xt_pool", bufs=4) as xt_pool,
                    tc.tile_pool(name="psB", bufs=1, space="PSUM") as psB_pool,
                    tc.tile_pool(name="psT", bufs=2, space="PSUM") as psT_pool,
                ):
                    for b in range(B):
                        for tq in range(2):
                            t0 = tq * 1024
                            ps = [psB_pool.tile([128, 512], F32, name=f"psB{i}",
                                                tag=f"psB{i}") for i in range(6)]
                            for kt in range(KT_D):
                                xt = xt_pool.tile([128, 1024], F32R, name="xt", tag="xt")
                                nc.sync.dma_start(
                                    out=xt[:],
                                    in_=xT[kt * 128:(kt + 1) * 128,
                                           b * S + t0: b * S + t0 + 1024])
                                for pi, wname in enumerate(("wq", "wk", "wv")):
                                    for nch in range(2):
                                        nc.tensor.matmul(
                                            ps[pi * 2 + nch][:],
                                            w_sb[wname][:, kt, :],
                                            xt[:, nch * 512:(nch + 1) * 512],
                                            start=(kt == 0), stop=(kt == KT_D - 1),
                                        )
                            for pi, (dname, bname) in enumerate(
                                    (("qT", "bq"), ("kT", "bk"), ("vT", "bv"))):
                                for nch in range(2):
                                    nc.vector.tensor_scalar_add(
                                        out=qkvT[dname][b][:, t0 + nch * 512:
                                                           t0 + (nch + 1) * 512],
                                        in0=ps[pi * 2 + nch][:],
                                        scalar1=b_sb[bname][:, 0:1],
                                    )
                        if "C" not in stages:
                            continue
                        # ---- stage C: V^T -> V_aug for batch b ----
                        for h in range(2):
                            pr = b * 2 + h
                            for g in range(2):  # groups of 8 ktiles
                                pst = psT_pool.tile([128, 512], F32R, name="pst", tag="pst")
                                for j in range(8):
                                    kt = g * 8 + j
                                    nc.tensor.transpose(
                                        out=pst[:, j * DH:(j + 1) * DH],
                                        in_=qkvT["vT"][b][h * DH:(h + 1) * DH,
                                                          kt * 128:(kt + 1) * 128],
                                        identity=ident[h * DH:(h + 1) * DH,
                                                       h * DH:(h + 1) * DH],
                                    )
                                nc.vector.tensor_copy(
                                    vaug[:, pr, g * 8:(g + 1) * 8, 0:DH],
                                    pst[:],
                                )
                if "D" not in stages:
                    continue
                # ------- stages D-G, pipelined per (batch, q-chunk) -------
                with (
                    tc.tile_pool(name="psS", bufs=2, space="PSUM") as psS_pool,
                    tc.tile_pool(name="psC", bufs=2, space="PSUM") as psC_pool,
                    tc.tile_pool(name="misc", bufs=2, space="PSUM") as misc_pool,
                    tc.tile_pool(name="exp_pool", bufs=4) as exp_pool,
                    tc.tile_pool(name="rpool", bufs=1) as rpool,
                    tc.tile_pool(name="fo_pool", bufs=4) as fo_pool,
                    tc.tile_pool(name="gy_pool", bufs=1) as gy_pool,
                ):
                    for b in range(B):
                        r_ts = []
                        for h in range(2):
                            # ---- stage D: attention for (batch b, head h) ----
                            pr = b * 2 + h
                            qT_h = qkvT["qT"][b][h * DH:(h + 1) * DH, :]
                            kT_h = qkvT["kT"][b][h * DH:(h + 1) * DH, :]
                            for qc in range(S // 512):
                                q0 = qc * 512
                                n_kt = q0 // 128 + 4
                                ps_ctx = psC_pool.tile([128, 512], F32, name="ps_ctx",
                                                       tag="ps_ctx")
                                for kg in range(n_kt // 2):
                                    ka, kb = 2 * kg, 2 * kg + 1
                                    offa = max(0, ka * 128 - q0)
                                    offb = max(0, kb * 128 - q0)
                                    ps_s = psS_pool.tile([128, 1024], F32, name="ps_s",
                                                         tag="ps_s")
                                    nc.tensor.matmul(
                                        ps_s[:, offa:512],
                                        kT_h[:, ka * 128:(ka + 1) * 128],
                                        qT_h[:, q0 + offa:q0 + 512],
                                        start=True, stop=True,
                                    )
                                    nc.tensor.matmul(
                                        ps_s[:, 512 + offb:1024],
                                        kT_h[:, kb * 128:(kb + 1) * 128],
                                        qT_h[:, q0 + offb:q0 + 512],
                                        start=True, stop=True,
                                    )
                                    ex = exp_pool.tile([128, 1024], F32R, name="ex", tag="ex")
                                    # one exp over both halves; the gap
                                    # [512:512+offb) holds stale-but-finite data
                                    # that the ctx matmuls never read.
                                    nc.scalar.activation(
                                        out=ex[:, offa:1024], in_=ps_s[:, offa:1024],
                                        func=mybir.ActivationFunctionType.Exp,
                                        scale=INV_SCALE,
                                    )
                                    # causal mask: multiplicative 0/1 on the
                                    # diagonal tiles, applied AFTER exp so DVE
                                    # stays off the PE->ACT critical path
                                    if ka * 128 >= q0:
                                        nc.vector.tensor_mul(
                                            out=ex[:, offa:offa + 128],
                                            in0=ex[:, offa:offa + 128],
                                            in1=tri01[:],
                                        )
                                    if kb * 128 >= q0:
                                        nc.vector.tensor_mul(
                                            out=ex[:, 512 + offb:512 + offb + 128],
                                            in0=ex[:, 512 + offb:512 + offb + 128],
                                            in1=tri01[:],
                                        )
                                    nc.tensor.matmul(
                                        ps_ctx[0:DH + 1, offa:512],
                                        vaug[:, pr, ka, :],
                                        ex[:, offa:512],
                                        start=(ka == 0), stop=False,
                                        skip_group_check=True,
                                    )
                                    nc.tensor.matmul(
                                        ps_ctx[0:DH + 1, offb:512],
                                        vaug[:, pr, kb, :],
                                        ex[:, 512 + offb:1024],
                                        start=False, stop=(kb == n_kt - 1),
                                        skip_group_check=True,
                                    )
                                nc.vector.tensor_copy(
                                    ctxu[0:DH + 1, pr, q0:q0 + 512],
                                    ps_ctx[0:DH + 1, :],
                                )
                            if "E" not in stages:
                                continue
                            # ---- stage E: r = 1/l = exp(-ln(l)) for this head ----
                            ln_f = rpool.tile([65, S], F32, name="ln_f", tag="ln_f")
                            nc.scalar.activation(
                                out=ln_f[64:65, :], in_=ctxu[64:65, pr, :],
                                func=mybir.ActivationFunctionType.Ln)
                            r_t = rpool.tile([65, S], F32R, name=f"r_t{h}", tag=f"r_t{h}")
                            nc.scalar.activation(
                                out=r_t[64:65, :], in_=ln_f[64:65, :],
                                func=mybir.ActivationFunctionType.Exp, scale=-1.0)
                            r_ts.append(r_t)
                        # ---- per q-chunk: normalize + partial out^T ----
                        if "F" not in stages or "E" not in stages:
                            continue
                        for qc in range(S // 512):
                            q0 = qc * 512
                            for h in range(2):
                                pr = b * 2 + h
                                bc = misc_pool.tile([128, 512], F32, name="bc", tag="efps")
                                nc.tensor.matmul(
                                    bc[0:DH, :],
                                    ones_r[64:65, 0:DH],
                                    r_ts[h][64:65, q0:q0 + 512],
                                    start=True, stop=True,
                                )
                                nc.vector.tensor_mul(
                                    out=ctxn[h * DH:(h + 1) * DH,
                                             b * S + q0: b * S + q0 + 512],
                                    in0=ctxu[0:DH, pr, q0:q0 + 512],
                                    in1=bc[0:DH, :],
                                )
                            for ot in range(KT_D):
                                ps_o = misc_pool.tile([128, 512], F32, name="ps_o",
                                                      tag="efps")
                                nc.tensor.matmul(
                                    ps_o[:],
                                    wo_sb[:, ot, :],
                                    ctxn[:, b * S + q0: b * S + q0 + 512],
                                    start=True, stop=True,
                                )
                                pot = fo_pool.tile([128, 512], PO_DT, name="pot", tag="pot")
                                if ot % 2 == 0:
                                    nc.vector.tensor_copy(pot[:], ps_o[:])
                                else:
                                    nc.scalar.activation(
                                        out=pot[:], in_=ps_o[:],
                                        func=mybir.ActivationFunctionType.Copy)
                                nc.sync.dma_start(
                                    out=po_b[b][ot * 128:(ot + 1) * 128, q0:q0 + 512],
                                    in_=pot[:])
                        if not do_collective:
                            continue
                        nc.gpsimd.collective_compute(
                            "ReduceScatter",
                            mybir.AluOpType.add,
                            ins=[po_b[b][:]],
                            outs=[rs_b[b][:]],
                            replica_groups=[list(range(NC))],
                        )
                        # ---- stage G: + bo, store batch b ----
                        yt_in = gy_pool.tile([128, S], PO_DT, name="yt_in", tag="yt_in")
                        nc.sync.dma_start(out=yt_in[:], in_=rs_b[b][:])
                        yo = gy_pool.tile([128, S], F32, name="yo", tag="yo")
                        nc.vector.tensor_scalar_add(
                            out=yo[:], in0=yt_in[:], scalar1=b_sb["bo"][:, 0:1])
                        nc.sync.dma_start(out=yT[:, b * S:(b + 1) * S], in_=yo[:])

    _split_waits(nc)
    return nc


def kernel(x, mask, Wq, bq, Wk, bk, Wv, bv, Wo, bo, trace=False, repeat=1, _in_maps_only=False):
    x = np.asarray(x, dtype=np.float32).reshape(T, D)
    xT = np.ascontiguousarray(x.T)
    in_maps = []
    for c in range(NC):
        sl = slice(c * HG, (c + 1) * HG)
        in_maps.append({
            "xT": xT,
            "wq": np.ascontiguousarray(np.asarray(Wq, np.float32)[:, sl]),
            "wk": np.ascontiguousarray(np.asarray(Wk, np.float32)[:, sl]),
            "wv": np.ascontiguousarray(np.asarray(Wv, np.float32)[:, sl]),
            "wo": np.ascontiguousarray(np.asarray(Wo, np.float32)[sl, :]),
            "bq": np.ascontiguousarray(np.asarray(bq, np.float32)[sl].reshape(HG, 1)),
            "bk": np.ascontiguousarray(np.asarray(bk, np.float32)[sl].reshape(HG, 1)),
            "bv": np.ascontiguousarray(np.asarray(bv, np.float32)[sl].reshape(HG, 1)),
            "bo": np.ascontiguousarray(np.asarray(bo, np.float32)[sl].reshape(HG, 1)),
        })
    if _in_maps_only:
        return in_maps
    nc = build_module(repeat=repeat)
    res = run_bass_kernel_spmd(nc, in_maps, core_ids=list(range(NC)), trace=trace)
    out = np.empty((T, D), dtype=np.float32)
    for c in range(NC):
        out[:, c * HG:(c + 1) * HG] = res.results[c]["yT"].T
    if trace:
        kernel.last_results = res
    return out.reshape(B, S, D)



# revision 7
# speedup vs baseline: 1.4925x; 1.4925x over previous
"""Multi-head causal self-attention (B=2, S=2048, D=1024, H=16) on 8 TRN2 cores.

Sharding: head-parallel. Core c owns head-group c = heads {2c, 2c+1}
(= 128 of the 1024 qkv dims, both batches).

Per core (all matmul operands bf16; PSUM accumulation stays f32):
  stage B: Q^T/K^T/V^T = (x @ W{q,k,v}[:, c-slice] + b)^T          [128, 4096]
  stage C: V^T -> V_aug [tok, 65] tiles (col 64 = ones, for the l-row trick)
  stage D: per (batch, q-chunk, head): scores^T = K^T.T-tiles @ Q^T (PE),
           exp (ACT, scale=1/8), causal mask (DVE, post-exp multiplicative),
           ctx^T accum (PE) with the ones column producing l = sum(exp).
  stage E: r = 1/l (DVE reciprocal), broadcast via PE outer product,
           normalize ctx^T -> bf16 chunk [128, 512].
  stage F: AllGather the normalized ctx chunk across cores ([128,512] ->
           [1024,512]), then out^T[c-cols, chunk] = Wo[:, c-slice].T-tiles @
           ctx_full + bo. The AG moves 8x less data than reduce-scattering
           partial outputs, and per-chunk AGs pipeline behind the next
           chunk's attention.

Host: passes x pre-transposed in bf16, weight column slices in bf16;
transposes y^T back.
"""

import sys

for p in ("/opt/trn_rl_repo", "/root/.axon_site/_ro/trn_rl_repo"):
    if p not in sys.path:
        sys.path.insert(0, p)

import numpy as np

import bass_rust
import concourse.bass as bass
import concourse.mybir as mybir
from concourse.bass_utils import run_bass_kernel_spmd
from concourse.masks import make_identity
from concourse.tile import TileContext

B, S, D = 2, 2048, 1024
H, DH = 16, 64
T = B * S              # 4096 tokens
NC = 8                 # cores
HG = D // NC           # 128 qkv dims per core (2 heads)
KT_D = D // 128        # 8 contraction tiles over d_model
QC = 512               # q-chunk width
NQC = S // QC          # 4 q-chunks per batch
INV_SCALE = 1.0 / float(np.sqrt(DH))  # 1/8
F32 = mybir.dt.float32
F32R = mybir.dt.float32r
BF16 = mybir.dt.bfloat16


def _split_waits(nc, max_waits=1):
    """This walrus build accepts one sync-wait per instruction; Tile sometimes
    emits more. Split extras into preceding NoOps on the same engine."""
    n = 0
    for f in nc.m.functions:
        for bb in f.blocks:
            out = []
            for inst in bb.instructions:
                si = getattr(inst, "sync_info", None)
                if si is not None and si.on_wait and len(si.on_wait) > max_waits:
                    waits = list(si.on_wait)
                    head, rest = waits[:-max_waits], waits[-max_waits:]
                    k = 0
                    while head:
                        chunk, head = head[:max_waits], head[max_waits:]
                        out.append(mybir.InstNoOp(
                            name=f"{inst.name}-wsplit-{k}", ins=[], outs=[],
                            engine=inst.engine,
                            sync_info=bass_rust.SyncInfo(on_wait=chunk, on_update=[]),
                        ))
                        k += 1
                    si.on_wait = rest
                    n += 1
                out.append(inst)
            bb.instructions = out
    return n


def build_module():
    nc = bass.Bass()

    xT = nc.dram_tensor("xT", [D, T], BF16, kind="ExternalInput")
    wq = nc.dram_tensor("wq", [D, HG], BF16, kind="ExternalInput")
    wk = nc.dram_tensor("wk", [D, HG], BF16, kind="ExternalInput")
    wv = nc.dram_tensor("wv", [D, HG], BF16, kind="ExternalInput")
    wo = nc.dram_tensor("wo", [D, HG], BF16, kind="ExternalInput")  # Wo[:, c-cols]
    bq = nc.dram_tensor("bq", [HG, 1], F32, kind="ExternalInput")
    bk = nc.dram_tensor("bk", [HG, 1], F32, kind="ExternalInput")
    bv = nc.dram_tensor("bv", [HG, 1], F32, kind="ExternalInput")
    bo = nc.dram_tensor("bo", [HG, 1], F32, kind="ExternalInput")
    yT = nc.dram_tensor("yT", [HG, T], F32, kind="ExternalOutput")

    # per-(batch, q-chunk) normalized-ctx chunk and its AllGather
    cg = [[nc.dram_tensor(f"cg{b}_{q}", [HG, QC], BF16) for q in range(NQC)]
          for b in range(B)]
    ag = [[nc.dram_tensor(f"ag{b}_{q}", [D, QC], BF16, addr_space="Shared")
           for q in range(NQC)] for b in range(B)]

    with TileContext(nc) as tc:
        with tc.tile_pool(name="persist", bufs=1) as pp:
            # weights as [128, kt, 128] (contraction tile on partitions)
            w_sb = {}
            for name, dram in (("wq", wq), ("wk", wk), ("wv", wv), ("wo", wo)):
                t = pp.tile([128, KT_D, HG], BF16, name=f"{name}_sb", tag=f"{name}_sb")
                nc.sync.dma_start(out=t[:], in_=dram[:].rearrange("(kt p) n -> p kt n", p=128))
                w_sb[name] = t
            b_sb = {}
            for name, dram in (("bq", bq), ("bk", bk), ("bv", bv), ("bo", bo)):
                t = pp.tile([HG, 1], F32, name=f"{name}_sb", tag=f"{name}_sb")
                nc.sync.dma_start(out=t[:], in_=dram[:])
                b_sb[name] = t

            # identity built in f32 (gpsimd memset can't write bf16 reliably),
            # then DVE-copied (rounds) into the bf16 tile transpose needs
            ident_f = pp.tile([128, 128], F32, name="ident_f", tag="ident_f")
            make_identity(nc, ident_f[:])
            ident = pp.tile([128, 128], BF16, name="ident", tag="ident")
            nc.vector.tensor_copy(ident[:], ident_f[:])
            # multiplicative causal mask for a diagonal 128x128 tile of
            # scores^T: keep [r, c] where r <= c (k <= q)
            tri_f = pp.tile([128, 128], F32, name="tri_f", tag="tri_f")
            nc.gpsimd.memset(tri_f[:], 1.0)
            nc.gpsimd.affine_select(
                out=tri_f[:], in_=tri_f[:],
                compare_op=mybir.AluOpType.is_ge, fill=0.0,
                base=0, pattern=[[1, 128]], channel_multiplier=-1,
            )
            tri01 = pp.tile([128, 128], BF16, name="tri01", tag="tri01")
            nc.vector.tensor_copy(tri01[:], tri_f[:])
            # ones row at partition 64 (base partition of the l row)
            ones_f = pp.tile([65, DH], F32, name="ones_f", tag="ones_f")
            nc.vector.memset(ones_f[:], 1.0)
            ones_r = pp.tile([65, DH], F32R, name="ones_r", tag="ones_r")
            nc.vector.tensor_copy(ones_r[:], ones_f[:])
            ones128 = pp.tile([128, B * 2 * (S // 128)], F32, name="ones128",
                              tag="ones128")
            nc.vector.memset(ones128[:], 1.0)

            qkvT = {}
            for name in ("qT", "kT", "vT"):
                qkvT[name] = [pp.tile([128, S], BF16, name=f"{name}{b}", tag=f"{name}{b}")
                              for b in range(B)]

            vaug = pp.tile([128, B * 2, S // 128, DH + 1], BF16, name="vaug", tag="vaug")
            nc.vector.tensor_copy(vaug[:, :, :, DH:DH + 1], ones128[:, :])
            # [65 used partitions, pair, q]; row 64 = l
            ctxu = pp.tile([128, B * 2, S], F32, name="ctxu", tag="ctxu")

            # ---------------- stage B+C: QKV projections ----------------
            with (
                tc.tile_pool(name="xt_pool", bufs=4) as xt_pool,
                tc.tile_pool(name="psB", bufs=1, space="PSUM") as psB_pool,
                tc.tile_pool(name="psT", bufs=2, space="PSUM") as psT_pool,
            ):
                for b in range(B):
                    for tq in range(2):
                        t0 = tq * 1024
                        ps = [psB_pool.tile([128, 512], F32, name=f"psB{i}",
                                            tag=f"psB{i}") for i in range(6)]
                        for kt in range(KT_D):
                            xt = xt_pool.tile([128, 1024], BF16, name="xt", tag="xt")
                            nc.sync.dma_start(
                                out=xt[:],
                                in_=xT[kt * 128:(kt + 1) * 128,
                                       b * S + t0: b * S + t0 + 1024])
                            for pi, wname in enumerate(("wq", "wk", "wv")):
                                for nch in range(2):
                                    nc.tensor.matmul(
                                        ps[pi * 2 + nch][:],
                                        w_sb[wname][:, kt, :],
                                        xt[:, nch * 512:(nch + 1) * 512],
                                        start=(kt == 0), stop=(kt == KT_D - 1),
                                    )
                        for pi, (dname, bname) in enumerate(
                                (("qT", "bq"), ("kT", "bk"), ("vT", "bv"))):
                            for nch in range(2):
                                nc.vector.tensor_scalar_add(
                                    out=qkvT[dname][b][:, t0 + nch * 512:
                                                       t0 + (nch + 1) * 512],
                                    in0=ps[pi * 2 + nch][:],
                                    scalar1=b_sb[bname][:, 0:1],
                                )
                    # ---- stage C: V^T -> V_aug for batch b ----
                    for h in range(2):
                        pr = b * 2 + h
                        for g in range(2):  # groups of 8 ktiles
                            pst = psT_pool.tile([128, 512], BF16, name="pst", tag="pst")
                            for j in range(8):
                                kt = g * 8 + j
                                nc.tensor.transpose(
                                    out=pst[:, j * DH:(j + 1) * DH],
                                    in_=qkvT["vT"][b][h * DH:(h + 1) * DH,
                                                      kt * 128:(kt + 1) * 128],
                                    identity=ident[h * DH:(h + 1) * DH,
                                                   h * DH:(h + 1) * DH],
                                )
                            nc.vector.tensor_copy(
                                vaug[:, pr, g * 8:(g + 1) * 8, 0:DH],
                                pst[:],
                            )

            # ------- stages D-F, pipelined per (batch, q-chunk) -------
            with (
                tc.tile_pool(name="psS", bufs=2, space="PSUM") as psS_pool,
                tc.tile_pool(name="psC", bufs=2, space="PSUM") as psC_pool,
                tc.tile_pool(name="misc", bufs=2, space="PSUM") as misc_pool,
                tc.tile_pool(name="exp_pool", bufs=4) as exp_pool,
                tc.tile_pool(name="rpool", bufs=2) as rpool,
                tc.tile_pool(name="cn_pool", bufs=2) as cn_pool,
                tc.tile_pool(name="cg_pool", bufs=2) as cg_pool,
                tc.tile_pool(name="yo_pool", bufs=2) as yo_pool,
            ):
                chunks = [(b, qc) for b in range(B) for qc in range(NQC)]

                def emit_outproj(b, qc):
                    # F': out^T chunk = Wo[:, c-slice].T-tiles @ gathered ctx
                    q0 = qc * QC
                    ctxg = cg_pool.tile([128, KT_D, QC], BF16, name="ctxg", tag="ctxg")
                    nc.gpsimd.dma_start(
                        out=ctxg[:],
                        in_=ag[b][qc][:].rearrange("(kt p) n -> p kt n", p=128))
                    ps_o = misc_pool.tile([128, QC], F32, name="ps_o", tag="mps")
                    for kt in range(KT_D):
                        nc.tensor.matmul(
                            ps_o[:],
                            w_sb["wo"][:, kt, :],
                            ctxg[:, kt, :],
                            start=(kt == 0), stop=(kt == KT_D - 1),
                        )
                    yo = yo_pool.tile([128, QC], F32, name="yo", tag="yo")
                    nc.vector.tensor_scalar_add(
                        out=yo[:], in0=ps_o[:], scalar1=b_sb["bo"][:, 0:1])
                    nc.sync.dma_start(
                        out=yT[:, b * S + q0: b * S + q0 + QC], in_=yo[:])

                for ci, (b, qc) in enumerate(chunks):
                    q0 = qc * QC
                    n_kt = q0 // 128 + 4
                    cn = cn_pool.tile([128, QC], BF16, name="cn", tag="cn")
                    for h in range(2):
                        # ---- stage D: attention for (b, qc, h) ----
                        pr = b * 2 + h
                        qT_h = qkvT["qT"][b][h * DH:(h + 1) * DH, :]
                        kT_h = qkvT["kT"][b][h * DH:(h + 1) * DH, :]
                        ps_ctx = psC_pool.tile([128, QC], F32, name="ps_ctx",
                                               tag="ps_ctx")
                        for kg in range(n_kt // 2):
                            ka, kb = 2 * kg, 2 * kg + 1
                            offa = max(0, ka * 128 - q0)
                            offb = max(0, kb * 128 - q0)
                            ps_s = psS_pool.tile([128, 1024], F32, name="ps_s",
                                                 tag="ps_s")
                            nc.tensor.matmul(
                                ps_s[:, offa:512],
                                kT_h[:, ka * 128:(ka + 1) * 128],
                                qT_h[:, q0 + offa:q0 + 512],
                                start=True, stop=True,
                            )
                            nc.tensor.matmul(
                                ps_s[:, 512 + offb:1024],
                                kT_h[:, kb * 128:(kb + 1) * 128],
                                qT_h[:, q0 + offb:q0 + 512],
                                start=True, stop=True,
                            )
                            ex = exp_pool.tile([128, 1024], BF16, name="ex", tag="ex")
                            # one exp over both halves; the gap
                            # [512:512+offb) holds stale-but-finite data
                            # that the ctx matmuls never read.
                            nc.scalar.activation(
                                out=ex[:, offa:1024], in_=ps_s[:, offa:1024],
                                func=mybir.ActivationFunctionType.Exp,
                                scale=INV_SCALE,
                            )
                            # causal mask: multiplicative 0/1 on the
                            # diagonal tiles, applied AFTER exp so DVE
                            # stays off the PE->ACT critical path
                            if ka * 128 >= q0:
                                nc.vector.tensor_mul(
                                    out=ex[:, offa:offa + 128],
                                    in0=ex[:, offa:offa + 128],
                                    in1=tri01[:],
                                )
                            if kb * 128 >= q0:
                                nc.vector.tensor_mul(
                                    out=ex[:, 512 + offb:512 + offb + 128],
                                    in0=ex[:, 512 + offb:512 + offb + 128],
                                    in1=tri01[:],
                                )
                            nc.tensor.matmul(
                                ps_ctx[0:DH + 1, offa:512],
                                vaug[:, pr, ka, :],
                                ex[:, offa:512],
                                start=(ka == 0), stop=False,
                                skip_group_check=True,
                            )
                            nc.tensor.matmul(
                                ps_ctx[0:DH + 1, offb:512],
                                vaug[:, pr, kb, :],
                                ex[:, 512 + offb:1024],
                                start=False, stop=(kb == n_kt - 1),
                                skip_group_check=True,
                            )
                        nc.vector.tensor_copy(
                            ctxu[0:DH + 1, pr, q0:q0 + 512],
                            ps_ctx[0:DH + 1, :],
                        )
                        # ---- stage E: r = 1/l, broadcast, normalize ----
                        r_t = rpool.tile([65, QC], F32R, name="r_t", tag="r_t")
                        # f32r out is bitwise f32 — no actual precision loss
                        with nc.allow_low_precision(reason="f32r == f32 bits"):
                            nc.vector.reciprocal(
                                out=r_t[64:65, :], in_=ctxu[64:65, pr, q0:q0 + QC])
                        bc = misc_pool.tile([128, QC], F32, name="bc", tag="mps")
                        nc.tensor.matmul(
                            bc[0:DH, :],
                            ones_r[64:65, 0:DH],
                            r_t[64:65, :],
                            start=True, stop=True,
                        )
                        nc.vector.tensor_mul(
                            out=cn[h * DH:(h + 1) * DH, :],
                            in0=ctxu[0:DH, pr, q0:q0 + QC],
                            in1=bc[0:DH, :],
                        )
                    # chunk ready: store + AllGather (gpsimd queue pipelines
                    # cg-store -> AG -> ctxg-load without touching PE/ACT)
                    nc.gpsimd.dma_start(out=cg[b][qc][:], in_=cn[:])
                    nc.gpsimd.collective_compute(
                        "AllGather",
                        mybir.AluOpType.bypass,
                        ins=[cg[b][qc][:]],
                        outs=[ag[b][qc][:]],
                        replica_groups=[list(range(NC))],
                    )
                    # emit the PREVIOUS chunk's output projection here so its
                    # PE work lands behind this chunk's attention, giving the
                    # AG time to complete without stalling the PE queue
                    if ci >= 1:
                        pb, pq = chunks[ci - 1]
                        emit_outproj(pb, pq)
                emit_outproj(*chunks[-1])

    _split_waits(nc)
    return nc


def kernel(x, mask, Wq, bq, Wk, bk, Wv, bv, Wo, bo, trace=False):
    import ml_dtypes
    bf16 = ml_dtypes.bfloat16
    x = np.asarray(x, dtype=np.float32).reshape(T, D)
    xT = np.ascontiguousarray(x.T).astype(bf16)
    in_maps = []
    for c in range(NC):
        sl = slice(c * HG, (c + 1) * HG)
        in_maps.append({
            "xT": xT,
            "wq": np.ascontiguousarray(np.asarray(Wq, np.float32)[:, sl]).astype(bf16),
            "wk": np.ascontiguousarray(np.asarray(Wk, np.float32)[:, sl]).astype(bf16),
            "wv": np.ascontiguousarray(np.asarray(Wv, np.float32)[:, sl]).astype(bf16),
            "wo": np.ascontiguousarray(np.asarray(Wo, np.float32)[:, sl]).astype(bf16),
            "bq": np.ascontiguousarray(np.asarray(bq, np.float32)[sl].reshape(HG, 1)),
            "bk": np.ascontiguousarray(np.asarray(bk, np.float32)[sl].reshape(HG, 1)),
            "bv": np.ascontiguousarray(np.asarray(bv, np.float32)[sl].reshape(HG, 1)),
            "bo": np.ascontiguousarray(np.asarray(bo, np.float32)[sl].reshape(HG, 1)),
        })
    nc = build_module()
    res = run_bass_kernel_spmd(nc, in_maps, core_ids=list(range(NC)), trace=trace)
    out = np.empty((T, D), dtype=np.float32)
    for c in range(NC):
        out[:, c * HG:(c + 1) * HG] = res.results[c]["yT"].T
    if trace:
        kernel.last_results = res
    return out.reshape(B, S, D)


# revision 17
# speedup vs baseline: 1.6659x; 1.1161x over previous
"""Multi-head causal self-attention (B=2, S=2048, D=1024, H=16) on 8 TRN2 cores.

Sharding: head-parallel for QKV+attention (core c owns heads {2c, 2c+1}),
token-parallel for the output projection (core c owns tokens
[256c, 256c+256) of each batch), bridged by a per-batch AllToAll of the
normalized context — 8x less wire traffic than gathering or
reduce-scattering, since nothing is replicated.

Per core (matmul operands bf16; PSUM accumulation f32):
  stage B: Q^T/K^T/V^T = (x @ W{q,k,v}[:, c-slice] + b)^T   [128, 4096]
           512-token chunks, double-buffered PSUM accumulators.
  stage C: V^T -> V_aug [tok, 65] tiles (col 64 = ones -> l row).
  stage D: per (batch, q-chunk, head): scores^T = K^T.T @ Q^T tiles (PE),
           exp (ACT, scale=1/8, diagonal tiles packed to skip masked cols),
           causal mask (DVE, post-exp multiplicative), ctx^T accum (PE).
  stage E: r = 1/l (DVE reciprocal_approx_fast), PE outer-product broadcast,
           normalize -> bf16. Emitted one h-slot late so the PE queue never
           waits on the reciprocal.
  stage F: per batch: AllToAll ctx chunks (head-sharded -> token-sharded),
           then out = Wo^T-tiles @ ctx_full + bo for this core's 256
           tokens/batch, using the full Wo.

Host: x pre-transposed bf16; Wq/Wk/Wv column slices bf16; full Wo/bo;
output reassembled token-wise from yT2 [1024, 2*256].
"""

import sys

for p in ("/opt/trn_rl_repo", "/root/.axon_site/_ro/trn_rl_repo"):
    if p not in sys.path:
        sys.path.insert(0, p)

import numpy as np

import bass_rust
import concourse.bass as bass
import concourse.mybir as mybir
from concourse.bass_utils import run_bass_kernel_spmd
from concourse.masks import make_identity
from concourse.tile import TileContext

B, S, D = 2, 2048, 1024
H, DH = 16, 64
T = B * S              # 4096 tokens
NC = 8                 # cores
HG = D // NC           # 128 qkv dims per core (2 heads)
KT_D = D // 128        # 8 contraction tiles over d_model
QC = 512               # q-chunk width
NQC = S // QC          # 4 q-chunks per batch
TPC = S // NC          # 256 tokens per core per batch (out-proj sharding)
INV_SCALE = 1.0 / float(np.sqrt(DH))  # 1/8
F32 = mybir.dt.float32
F32R = mybir.dt.float32r
BF16 = mybir.dt.bfloat16


def _split_waits(nc, max_waits=1):
    """This walrus build accepts one sync-wait per instruction; Tile sometimes
    emits more. Split extras into preceding NoOps on the same engine."""
    n = 0
    for f in nc.m.functions:
        for bb in f.blocks:
            out = []
            for inst in bb.instructions:
                si = getattr(inst, "sync_info", None)
                if si is not None and si.on_wait and len(si.on_wait) > max_waits:
                    waits = list(si.on_wait)
                    head, rest = waits[:-max_waits], waits[-max_waits:]
                    k = 0
                    while head:
                        chunk, head = head[:max_waits], head[max_waits:]
                        out.append(mybir.InstNoOp(
                            name=f"{inst.name}-wsplit-{k}", ins=[], outs=[],
                            engine=inst.engine,
                            sync_info=bass_rust.SyncInfo(on_wait=chunk, on_update=[]),
                        ))
                        k += 1
                    si.on_wait = rest
                    n += 1
                out.append(inst)
            bb.instructions = out
    return n


def build_module():
    nc = bass.Bass()

    xT = nc.dram_tensor("xT", [D, T], BF16, kind="ExternalInput")
    wq = nc.dram_tensor("wq", [D, HG], BF16, kind="ExternalInput")
    wk = nc.dram_tensor("wk", [D, HG], BF16, kind="ExternalInput")
    wv = nc.dram_tensor("wv", [D, HG], BF16, kind="ExternalInput")
    wo = nc.dram_tensor("wo", [D, D], BF16, kind="ExternalInput")  # full Wo
    bq = nc.dram_tensor("bq", [HG, 1], F32, kind="ExternalInput")
    bk = nc.dram_tensor("bk", [HG, 1], F32, kind="ExternalInput")
    bv = nc.dram_tensor("bv", [HG, 1], F32, kind="ExternalInput")
    bo = nc.dram_tensor("bo", [D, 1], F32, kind="ExternalInput")   # full bo
    # output: this core's TPC tokens of each batch, all D dims
    yT2 = nc.dram_tensor("yT2", [D, B * TPC], F32, kind="ExternalOutput")

    # AllToAll buffers: [token-group/src-rank, 128, TPC]
    a2a_in = [nc.dram_tensor(f"a2i{b}", [NC, HG, TPC], BF16) for b in range(B)]
    a2a_out = [nc.dram_tensor(f"a2o{b}", [NC, HG, TPC], BF16) for b in range(B)]

    with TileContext(nc) as tc:
        with tc.tile_pool(name="persist", bufs=1) as pp:
            # qkv weight col-slices as [128, kt, 128]
            w_sb = {}
            for name, dram in (("wq", wq), ("wk", wk), ("wv", wv)):
                t = pp.tile([128, KT_D, HG], BF16, name=f"{name}_sb", tag=f"{name}_sb")
                nc.sync.dma_start(out=t[:], in_=dram[:].rearrange("(kt p) n -> p kt n", p=128))
                w_sb[name] = t
            # full Wo as [128, kt, 1024] — gpsimd queue so the 2MB load does
            # not head-of-line block the first xt loads on the sync queue
            wo_sb = pp.tile([128, KT_D, D], BF16, name="wo_sb", tag="wo_sb")
            nc.gpsimd.dma_start(out=wo_sb[:], in_=wo[:].rearrange("(kt p) n -> p kt n", p=128))
            b_sb = {}
            for name, dram in (("bq", bq), ("bk", bk), ("bv", bv)):
                t = pp.tile([HG, 1], F32, name=f"{name}_sb", tag=f"{name}_sb")
                nc.sync.dma_start(out=t[:], in_=dram[:])
                b_sb[name] = t
            bo_sb = pp.tile([128, KT_D], F32, name="bo_sb", tag="bo_sb")
            nc.sync.dma_start(out=bo_sb[:], in_=bo[:].rearrange("(ot p) one -> p (ot one)", p=128))

            ident_f = pp.tile([128, 128], F32, name="ident_f", tag="ident_f")
            make_identity(nc, ident_f[:])
            ident = pp.tile([128, 128], BF16, name="ident", tag="ident")
            nc.vector.tensor_copy(ident[:], ident_f[:])
            # multiplicative causal mask for a diagonal 128x128 tile of
            # scores^T: keep [r, c] where r <= c (k <= q)
            tri_f = pp.tile([128, 128], F32, name="tri_f", tag="tri_f")
            nc.gpsimd.memset(tri_f[:], 1.0)
            nc.gpsimd.affine_select(
                out=tri_f[:], in_=tri_f[:],
                compare_op=mybir.AluOpType.is_ge, fill=0.0,
                base=0, pattern=[[1, 128]], channel_multiplier=-1,
            )
            tri01 = pp.tile([128, 128], BF16, name="tri01", tag="tri01")
            nc.vector.tensor_copy(tri01[:], tri_f[:])
            # ones row at partition 64 (base partition of the l row)
            ones_f = pp.tile([65, DH], F32, name="ones_f", tag="ones_f")
            nc.vector.memset(ones_f[:], 1.0)
            ones_r = pp.tile([65, DH], F32R, name="ones_r", tag="ones_r")
            nc.vector.tensor_copy(ones_r[:], ones_f[:])
            ones128 = pp.tile([128, B * 2 * (S // 128)], F32, name="ones128",
                              tag="ones128")
            nc.vector.memset(ones128[:], 1.0)

            qkvT = {}
            for name in ("qT", "kT", "vT"):
                qkvT[name] = [pp.tile([128, S], BF16, name=f"{name}{b}", tag=f"{name}{b}")
                              for b in range(B)]

            vaug = pp.tile([128, B * 2, S // 128, DH + 1], BF16, name="vaug", tag="vaug")
            nc.vector.tensor_copy(vaug[:, :, :, DH:DH + 1], ones128[:, :])
            # [65 used partitions, pair, q]; row 64 = l
            ctxu = pp.tile([128, B * 2, S], F32, name="ctxu", tag="ctxu")

            # ---------------- stage B+C: QKV projections ----------------
            with (
                tc.tile_pool(name="xt_pool", bufs=4) as xt_pool,
                tc.tile_pool(name="psB", bufs=2, space="PSUM") as psB_pool,
                tc.tile_pool(name="psT", bufs=2, space="PSUM") as psT_pool,
            ):
                for b in range(B):
                    for ch in range(S // 512):
                        t0 = ch * 512
                        ps = [psB_pool.tile([128, 512], F32, name=f"psB{i}",
                                            tag=f"psB{i}") for i in range(3)]
                        for kt in range(KT_D):
                            xt = xt_pool.tile([128, 512], BF16, name="xt", tag="xt")
                            nc.sync.dma_start(
                                out=xt[:],
                                in_=xT[kt * 128:(kt + 1) * 128,
                                       b * S + t0: b * S + t0 + 512])
                            for pi, wname in enumerate(("wq", "wk", "wv")):
                                nc.tensor.matmul(
                                    ps[pi][:],
                                    w_sb[wname][:, kt, :],
                                    xt[:],
                                    start=(kt == 0), stop=(kt == KT_D - 1),
                                )
                        for pi, (dname, bname) in enumerate(
                                (("qT", "bq"), ("kT", "bk"), ("vT", "bv"))):
                            nc.vector.tensor_scalar_add(
                                out=qkvT[dname][b][:, t0:t0 + 512],
                                in0=ps[pi][:],
                                scalar1=b_sb[bname][:, 0:1],
                            )
                    # ---- stage C: V^T -> V_aug for batch b ----
                    for h in range(2):
                        pr = b * 2 + h
                        for g in range(2):  # groups of 8 ktiles
                            pst = psT_pool.tile([128, 512], BF16, name="pst", tag="pst")
                            for j in range(8):
                                kt = g * 8 + j
                                nc.tensor.transpose(
                                    out=pst[:, j * DH:(j + 1) * DH],
                                    in_=qkvT["vT"][b][h * DH:(h + 1) * DH,
                                                      kt * 128:(kt + 1) * 128],
                                    identity=ident[h * DH:(h + 1) * DH,
                                                   h * DH:(h + 1) * DH],
                                )
                            nc.vector.tensor_copy(
                                vaug[:, pr, g * 8:(g + 1) * 8, 0:DH],
                                pst[:],
                            )

            # ------- stages D-F, pipelined per (batch, q-chunk) -------
            with (
                tc.tile_pool(name="psS", bufs=2, space="PSUM") as psS_pool,
                tc.tile_pool(name="psC", bufs=1, space="PSUM") as psC_pool,
                tc.tile_pool(name="psBC", bufs=1, space="PSUM") as psBC_pool,
                tc.tile_pool(name="psO", bufs=2, space="PSUM") as psO_pool,
                tc.tile_pool(name="exp_pool", bufs=4) as exp_pool,
                tc.tile_pool(name="rpool", bufs=4) as rpool,
                tc.tile_pool(name="cn_pool", bufs=3) as cn_pool,
                tc.tile_pool(name="cf_pool", bufs=2) as cf_pool,
                tc.tile_pool(name="yo_pool", bufs=2) as yo_pool,
            ):
                # r tiles per (b, qc, h), produced during D, consumed by the
                # deferred bc/normalize
                r_tiles = {}
                cn_tiles = {}

                def emit_D(b, qc, h):
                    # attention for (b, qc, h); also emits ctx copy + recip
                    # (DVE/ACT work that never blocks the PE queue)
                    q0 = qc * QC
                    n_kt = q0 // 128 + 4
                    pr = b * 2 + h
                    qT_h = qkvT["qT"][b][h * DH:(h + 1) * DH, :]
                    kT_h = qkvT["kT"][b][h * DH:(h + 1) * DH, :]
                    ps_ctx = psC_pool.tile([128, QC], F32, name="ps_ctx",
                                           tag="ps_ctx")
                    for kg in range(n_kt // 2):
                        ka, kb = 2 * kg, 2 * kg + 1
                        offa = max(0, ka * 128 - q0)
                        offb = max(0, kb * 128 - q0)
                        ps_s = psS_pool.tile([128, 1024], F32, name="ps_s",
                                             tag="ps_s")
                        # kb's block is packed at column 512 (width 512-offb)
                        # so the exp range [offa:1024-offb] has no dead gap
                        nc.tensor.matmul(
                            ps_s[:, offa:512],
                            kT_h[:, ka * 128:(ka + 1) * 128],
                            qT_h[:, q0 + offa:q0 + 512],
                            start=True, stop=True,
                        )
                        nc.tensor.matmul(
                            ps_s[:, 512:1024 - offb],
                            kT_h[:, kb * 128:(kb + 1) * 128],
                            qT_h[:, q0 + offb:q0 + 512],
                            start=True, stop=True,
                        )
                        ex = exp_pool.tile([128, 1024], BF16, name="ex", tag="ex")
                        nc.scalar.activation(
                            out=ex[:, offa:1024 - offb], in_=ps_s[:, offa:1024 - offb],
                            func=mybir.ActivationFunctionType.Exp,
                            scale=INV_SCALE,
                        )
                        # causal mask: multiplicative 0/1 on the diagonal
                        # tiles, applied AFTER exp (off the PE->ACT path)
                        if ka * 128 >= q0:
                            nc.vector.tensor_mul(
                                out=ex[:, offa:offa + 128],
                                in0=ex[:, offa:offa + 128],
                                in1=tri01[:],
                            )
                        if kb * 128 >= q0:
                            nc.vector.tensor_mul(
                                out=ex[:, 512:640],
                                in0=ex[:, 512:640],
                                in1=tri01[:],
                            )
                        nc.tensor.matmul(
                            ps_ctx[0:DH + 1, offa:512],
                            vaug[:, pr, ka, :],
                            ex[:, offa:512],
                            start=(ka == 0), stop=False,
                            skip_group_check=True,
                        )
                        nc.tensor.matmul(
                            ps_ctx[0:DH + 1, offb:512],
                            vaug[:, pr, kb, :],
                            ex[:, 512:1024 - offb],
                            start=False, stop=(kb == n_kt - 1),
                            skip_group_check=True,
                        )
                    nc.vector.tensor_copy(
                        ctxu[0:DH + 1, pr, q0:q0 + 512],
                        ps_ctx[0:DH + 1, :],
                    )
                    # r = 1/l = exp(-ln(l)); ln/exp share the attention exp's
                    # ACT table (reciprocal would force table reloads), and
                    # the exp writes f32r (verifier-approved rounding op)
                    ln_f = rpool.tile([65, QC], F32, name="ln_f", tag="ln_f")
                    nc.scalar.activation(
                        out=ln_f[64:65, :], in_=ctxu[64:65, pr, q0:q0 + QC],
                        func=mybir.ActivationFunctionType.Ln)
                    r_t = rpool.tile([65, QC], F32R, name="r_t", tag="r_t")
                    nc.scalar.activation(
                        out=r_t[64:65, :], in_=ln_f[64:65, :],
                        func=mybir.ActivationFunctionType.Exp, scale=-1.0)
                    r_tiles[(b, qc, h)] = r_t

                def emit_bcnorm(b, qc, h):
                    # deferred: broadcast r along 64 partitions (PE outer
                    # product) + normalize ctx into the chunk's bf16 tile
                    q0 = qc * QC
                    pr = b * 2 + h
                    if h == 0:
                        cn_tiles[(b, qc)] = cn_pool.tile(
                            [128, QC], BF16, name="cn", tag="cn")
                    cn = cn_tiles[(b, qc)]
                    r_t = r_tiles.pop((b, qc, h))
                    bc = psBC_pool.tile([128, QC], F32, name="bc", tag="bc")
                    nc.tensor.matmul(
                        bc[0:DH, :],
                        ones_r[64:65, 0:DH],
                        r_t[64:65, :],
                        start=True, stop=True,
                    )
                    nc.vector.tensor_mul(
                        out=cn[h * DH:(h + 1) * DH, :],
                        in0=ctxu[0:DH, pr, q0:q0 + QC],
                        in1=bc[0:DH, :],
                    )
                    if h == 1:
                        # chunk complete: ship to the A2A input buffer
                        # (token groups 2qc, 2qc+1)
                        nc.gpsimd.dma_start(
                            out=a2a_in[b][:].rearrange("g p n -> p g n")[
                                :, 2 * qc:2 * qc + 2, :],
                            in_=cn[:].rearrange("p (g n) -> p g n", g=2),
                        )
                        del cn_tiles[(b, qc)]

                def emit_a2a(b):
                    nc.gpsimd.collective_compute(
                        "AllToAll",
                        mybir.AluOpType.bypass,
                        ins=[a2a_in[b][:]],
                        outs=[a2a_out[b][:]],
                        replica_groups=[list(range(NC))],
                    )

                ctxf_tiles = {}

                def emit_ctxf_load(b):
                    ctxf = cf_pool.tile([128, KT_D, TPC], BF16, name="ctxf",
                                        tag="ctxf")
                    nc.gpsimd.dma_start(
                        out=ctxf[:],
                        in_=a2a_out[b][:].rearrange("kt p n -> p kt n"))
                    ctxf_tiles[b] = ctxf

                def emit_outproj(b):
                    # token-sharded output projection for this core's TPC
                    # tokens of batch b, using the full Wo
                    if b not in ctxf_tiles:
                        emit_ctxf_load(b)
                    ctxf = ctxf_tiles.pop(b)
                    for og in range(KT_D // 2):
                        # two out-dim tiles share one PSUM bank
                        ps_o = psO_pool.tile([128, 2, TPC], F32, name="ps_o",
                                             tag="ps_o")
                        for sub in range(2):
                            ot = og * 2 + sub
                            for kt in range(KT_D):
                                nc.tensor.matmul(
                                    ps_o[:, sub, :],
                                    wo_sb[:, kt, ot * 128:(ot + 1) * 128],
                                    ctxf[:, kt, :],
                                    start=(kt == 0), stop=(kt == KT_D - 1),
                                    skip_group_check=True,
                                )
                        yo = yo_pool.tile([128, 2, TPC], F32, name="yo", tag="yo")
                        for sub in range(2):
                            ot = og * 2 + sub
                            nc.vector.tensor_scalar_add(
                                out=yo[:, sub, :], in0=ps_o[:, sub, :],
                                scalar1=bo_sb[:, ot:ot + 1],
                            )
                        nc.sync.dma_start(
                            out=yT2[og * 256:(og + 1) * 256,
                                    b * TPC:(b + 1) * TPC].rearrange(
                                "(ot p) n -> p ot n", p=128),
                            in_=yo[:],
                        )

                # pipeline: bc/normalize of (qc, h) is emitted one h-slot
                # late so the PE queue never waits on the reciprocal chain;
                # batch b's A2A+outproj hide under batch b+1's attention.
                pending = []
                for b in range(B):
                    for qc in range(NQC):
                        emit_D(b, qc, 0)
                        if pending:
                            pending.pop(0)()
                        emit_D(b, qc, 1)
                        emit_bcnorm(b, qc, 0)
                        pending.append(
                            (lambda bb=b, qq=qc: (
                                emit_bcnorm(bb, qq, 1),
                                emit_a2a(bb) if qq == NQC - 1 else None)))
                        if b == 1 and qc == 1:
                            # b0's gathered-ctx load, well after its A2A and
                            # before b1's A2A occupies the gpsimd queue
                            pending.append(lambda: emit_ctxf_load(0))
                for fn in pending:
                    fn()
                # batch 0's out-proj here: its PE work fills the bubble while
                # batch 1's A2A is in flight; batch 1's follows
                emit_outproj(0)
                emit_outproj(1)

    _split_waits(nc)
    return nc


def _r(ap):
    return ap.bitcast(F32R)


def kernel(x, mask, Wq, bq, Wk, bk, Wv, bv, Wo, bo, trace=False):
    import ml_dtypes
    bf16 = ml_dtypes.bfloat16
    x = np.asarray(x, dtype=np.float32).reshape(T, D)
    xT = np.ascontiguousarray(x.T).astype(bf16)
    Wo_bf = np.ascontiguousarray(np.asarray(Wo, np.float32)).astype(bf16)
    bo_f = np.ascontiguousarray(np.asarray(bo, np.float32).reshape(D, 1))
    in_maps = []
    for c in range(NC):
        sl = slice(c * HG, (c + 1) * HG)
        in_maps.append({
            "xT": xT,
            "wq": np.ascontiguousarray(np.asarray(Wq, np.float32)[:, sl]).astype(bf16),
            "wk": np.ascontiguousarray(np.asarray(Wk, np.float32)[:, sl]).astype(bf16),
            "wv": np.ascontiguousarray(np.asarray(Wv, np.float32)[:, sl]).astype(bf16),
            "wo": Wo_bf,
            "bq": np.ascontiguousarray(np.asarray(bq, np.float32)[sl].reshape(HG, 1)),
            "bk": np.ascontiguousarray(np.asarray(bk, np.float32)[sl].reshape(HG, 1)),
            "bv": np.ascontiguousarray(np.asarray(bv, np.float32)[sl].reshape(HG, 1)),
            "bo": bo_f,
        })
    nc = build_module()
    res = run_bass_kernel_spmd(nc, in_maps, core_ids=list(range(NC)), trace=trace)
    out = np.empty((B, S, D), dtype=np.float32)
    for c in range(NC):
        y = res.results[c]["yT2"]  # [D, B*TPC]
        for b in range(B):
            out[b, c * TPC:(c + 1) * TPC, :] = y[:, b * TPC:(b + 1) * TPC].T
    if trace:
        kernel.last_results = res
    return out.reshape(B, S, D)


# revision 23
# speedup vs baseline: 1.6922x; 1.0158x over previous
"""Multi-head causal self-attention (B=2, S=2048, D=1024, H=16) on 8 TRN2 cores.

Sharding: head-parallel for QKV+attention (core c owns heads {2c, 2c+1}),
token-parallel for the output projection (core c owns tokens
[256c, 256c+256) of each batch), bridged by a per-batch AllToAll of the
normalized context — 8x less wire traffic than gathering or
reduce-scattering, since nothing is replicated.

Per core (matmul operands bf16; PSUM accumulation f32):
  stage B: Q^T/K^T/V^T = (x @ W{q,k,v}[:, c-slice] + b)^T   [128, 4096]
           512-token chunks, double-buffered PSUM accumulators.
  stage C: V^T -> V_aug [tok, 65] tiles (col 64 = ones -> l row).
  stage D: per (batch, q-chunk, head): scores^T = K^T.T @ Q^T tiles (PE),
           exp (ACT, scale=1/8, diagonal tiles packed to skip masked cols),
           causal mask (DVE, post-exp multiplicative), ctx^T accum (PE).
  stage E: r = 1/l (DVE reciprocal_approx_fast), PE outer-product broadcast,
           normalize -> bf16. Emitted one h-slot late so the PE queue never
           waits on the reciprocal.
  stage F: per batch: AllToAll ctx chunks (head-sharded -> token-sharded),
           then out = Wo^T-tiles @ ctx_full + bo for this core's 256
           tokens/batch, using the full Wo.

Host: x pre-transposed bf16; Wq/Wk/Wv column slices bf16; full Wo/bo;
output reassembled token-wise from yT2 [1024, 2*256].
"""

import sys

for p in ("/opt/trn_rl_repo", "/root/.axon_site/_ro/trn_rl_repo"):
    if p not in sys.path:
        sys.path.insert(0, p)

import numpy as np

import bass_rust
import concourse.bass as bass
import concourse.mybir as mybir
from concourse.bass_utils import run_bass_kernel_spmd
from concourse.masks import make_identity
from concourse.tile import TileContext

B, S, D = 2, 2048, 1024
H, DH = 16, 64
T = B * S              # 4096 tokens
NC = 8                 # cores
HG = D // NC           # 128 qkv dims per core (2 heads)
KT_D = D // 128        # 8 contraction tiles over d_model
QC = 512               # q-chunk width
NQC = S // QC          # 4 q-chunks per batch
TPC = S // NC          # 256 tokens per core per batch (out-proj sharding)
INV_SCALE = 1.0 / float(np.sqrt(DH))  # 1/8
F32 = mybir.dt.float32
F32R = mybir.dt.float32r
BF16 = mybir.dt.bfloat16


def _split_waits(nc, max_waits=1):
    """This walrus build accepts one sync-wait per instruction; Tile sometimes
    emits more. Split extras into preceding NoOps on the same engine."""
    n = 0
    for f in nc.m.functions:
        for bb in f.blocks:
            out = []
            for inst in bb.instructions:
                si = getattr(inst, "sync_info", None)
                if si is not None and si.on_wait and len(si.on_wait) > max_waits:
                    waits = list(si.on_wait)
                    head, rest = waits[:-max_waits], waits[-max_waits:]
                    k = 0
                    while head:
                        chunk, head = head[:max_waits], head[max_waits:]
                        out.append(mybir.InstNoOp(
                            name=f"{inst.name}-wsplit-{k}", ins=[], outs=[],
                            engine=inst.engine,
                            sync_info=bass_rust.SyncInfo(on_wait=chunk, on_update=[]),
                        ))
                        k += 1
                    si.on_wait = rest
                    n += 1
                out.append(inst)
            bb.instructions = out
    return n


def build_module():
    nc = bass.Bass()

    # weights arrive host-pre-tiled ([p, kt, n] flattened) so the loads are
    # fully contiguous DMAs instead of 256B-descriptor gather patterns
    xT = nc.dram_tensor("xT", [D, T], BF16, kind="ExternalInput")
    wq = nc.dram_tensor("wq", [128, KT_D * HG], BF16, kind="ExternalInput")
    wk = nc.dram_tensor("wk", [128, KT_D * HG], BF16, kind="ExternalInput")
    wv = nc.dram_tensor("wv", [128, KT_D * HG], BF16, kind="ExternalInput")
    wo = nc.dram_tensor("wo", [128, KT_D * D], BF16, kind="ExternalInput")
    bq = nc.dram_tensor("bq", [HG, 1], F32, kind="ExternalInput")
    bk = nc.dram_tensor("bk", [HG, 1], F32, kind="ExternalInput")
    bv = nc.dram_tensor("bv", [HG, 1], F32, kind="ExternalInput")
    bo = nc.dram_tensor("bo", [128, KT_D], F32, kind="ExternalInput")  # full bo
    # output: this core's TPC tokens of each batch, all D dims
    yT2 = nc.dram_tensor("yT2", [D, B * TPC], F32, kind="ExternalOutput")

    # AllToAll buffers: [token-group/src-rank, 128, TPC]
    a2a_in = [nc.dram_tensor(f"a2i{b}", [NC, HG, TPC], BF16) for b in range(B)]
    a2a_out = [nc.dram_tensor(f"a2o{b}", [NC, HG, TPC], BF16) for b in range(B)]

    with TileContext(nc) as tc:
        with tc.tile_pool(name="persist", bufs=1) as pp:
            # qkv weight col-slices as [128, kt, 128]
            w_sb = {}
            for name, dram in (("wq", wq), ("wk", wk), ("wv", wv)):
                t = pp.tile([128, KT_D, HG], BF16, name=f"{name}_sb", tag=f"{name}_sb")
                nc.sync.dma_start(out=t[:], in_=dram[:].rearrange("p (kt n) -> p kt n", n=HG))
                w_sb[name] = t
            # full Wo as [128, kt, 1024] — gpsimd queue so the 2MB load does
            # not head-of-line block the first xt loads on the sync queue
            wo_sb = pp.tile([128, KT_D, D], BF16, name="wo_sb", tag="wo_sb")
            nc.gpsimd.dma_start(out=wo_sb[:], in_=wo[:].rearrange("p (kt n) -> p kt n", n=D))
            b_sb = {}
            for name, dram in (("bq", bq), ("bk", bk), ("bv", bv)):
                t = pp.tile([HG, 1], F32, name=f"{name}_sb", tag=f"{name}_sb")
                nc.sync.dma_start(out=t[:], in_=dram[:])
                b_sb[name] = t
            bo_sb = pp.tile([128, KT_D], F32, name="bo_sb", tag="bo_sb")
            nc.gpsimd.dma_start(out=bo_sb[:], in_=bo[:])

            ident_f = pp.tile([128, 128], F32, name="ident_f", tag="ident_f")
            make_identity(nc, ident_f[:])
            ident = pp.tile([128, 128], BF16, name="ident", tag="ident")
            nc.vector.tensor_copy(ident[:], ident_f[:])
            # multiplicative causal mask for a diagonal 128x128 tile of
            # scores^T: keep [r, c] where r <= c (k <= q)
            tri_f = pp.tile([128, 128], F32, name="tri_f", tag="tri_f")
            nc.gpsimd.memset(tri_f[:], 1.0)
            nc.gpsimd.affine_select(
                out=tri_f[:], in_=tri_f[:],
                compare_op=mybir.AluOpType.is_ge, fill=0.0,
                base=0, pattern=[[1, 128]], channel_multiplier=-1,
            )
            tri01 = pp.tile([128, 128], BF16, name="tri01", tag="tri01")
            nc.vector.tensor_copy(tri01[:], tri_f[:])
            # ones row at partition 64 (base partition of the l row)
            ones_f = pp.tile([65, DH], F32, name="ones_f", tag="ones_f")
            nc.vector.memset(ones_f[:], 1.0)
            ones_r = pp.tile([65, DH], F32R, name="ones_r", tag="ones_r")
            nc.vector.tensor_copy(ones_r[:], ones_f[:])
            ones128 = pp.tile([128, B * 2 * (S // 128)], F32, name="ones128",
                              tag="ones128")
            nc.vector.memset(ones128[:], 1.0)

            qkvT = {}
            for name in ("qT", "kT", "vT"):
                qkvT[name] = [pp.tile([128, S], BF16, name=f"{name}{b}", tag=f"{name}{b}")
                              for b in range(B)]

            vaug = pp.tile([128, B * 2, S // 128, DH + 1], BF16, name="vaug", tag="vaug")
            nc.vector.tensor_copy(vaug[:, :, :, DH:DH + 1], ones128[:, :])
            # [65 used partitions, pair, q]; row 64 = l
            ctxu = pp.tile([128, B * 2, S], F32, name="ctxu", tag="ctxu")

            # ---------------- stage B+C: QKV projections ----------------
            with (
                tc.tile_pool(name="xt_pool", bufs=3) as xt_pool,
                tc.tile_pool(name="psB", bufs=1, space="PSUM") as psB_pool,
                tc.tile_pool(name="psT", bufs=2, space="PSUM") as psT_pool,
            ):
                for b in range(B):
                    for tq in range(2):
                        t0 = tq * 1024
                        # 6 accumulators: (3 projections) x (2 512-token
                        # halves of one 1024-wide x load)
                        ps = [psB_pool.tile([128, 512], F32, name=f"psB{i}",
                                            tag=f"psB{i}") for i in range(6)]
                        for kt in range(KT_D):
                            xt = xt_pool.tile([128, 1024], BF16, name="xt", tag="xt")
                            nc.sync.dma_start(
                                out=xt[:],
                                in_=xT[kt * 128:(kt + 1) * 128,
                                       b * S + t0: b * S + t0 + 1024])
                            for pi, wname in enumerate(("wq", "wk", "wv")):
                                for nch in range(2):
                                    nc.tensor.matmul(
                                        ps[pi * 2 + nch][:],
                                        w_sb[wname][:, kt, :],
                                        xt[:, nch * 512:(nch + 1) * 512],
                                        start=(kt == 0), stop=(kt == KT_D - 1),
                                    )
                        for pi, (dname, bname) in enumerate(
                                (("qT", "bq"), ("kT", "bk"), ("vT", "bv"))):
                            for nch in range(2):
                                nc.vector.tensor_scalar_add(
                                    out=qkvT[dname][b][:, t0 + nch * 512:
                                                       t0 + (nch + 1) * 512],
                                    in0=ps[pi * 2 + nch][:],
                                    scalar1=b_sb[bname][:, 0:1],
                                )
                    # ---- stage C: V^T -> V_aug for batch b ----
                    for h in range(2):
                        pr = b * 2 + h
                        for g in range(2):  # groups of 8 ktiles
                            pst = psT_pool.tile([128, 512], BF16, name="pst", tag="pst")
                            for j in range(8):
                                kt = g * 8 + j
                                nc.tensor.transpose(
                                    out=pst[:, j * DH:(j + 1) * DH],
                                    in_=qkvT["vT"][b][h * DH:(h + 1) * DH,
                                                      kt * 128:(kt + 1) * 128],
                                    identity=ident[h * DH:(h + 1) * DH,
                                                   h * DH:(h + 1) * DH],
                                )
                            nc.vector.tensor_copy(
                                vaug[:, pr, g * 8:(g + 1) * 8, 0:DH],
                                pst[:],
                            )

            # ------- stages D-F, pipelined per (batch, q-chunk) -------
            with (
                tc.tile_pool(name="psS", bufs=2, space="PSUM") as psS_pool,
                tc.tile_pool(name="psC", bufs=1, space="PSUM") as psC_pool,
                tc.tile_pool(name="psBC", bufs=1, space="PSUM") as psBC_pool,
                tc.tile_pool(name="psO", bufs=2, space="PSUM") as psO_pool,
                tc.tile_pool(name="exp_pool", bufs=4) as exp_pool,
                tc.tile_pool(name="rpool", bufs=4) as rpool,
                tc.tile_pool(name="cn_pool", bufs=4) as cn_pool,
                tc.tile_pool(name="cf_pool", bufs=2) as cf_pool,
                tc.tile_pool(name="yo_pool", bufs=2) as yo_pool,
            ):
                # r tiles per (b, qc, h), produced during D, consumed by the
                # deferred bc/normalize
                r_tiles = {}
                cn_tiles = {}

                def emit_D(b, qc, h):
                    # attention for (b, qc, h); also emits ctx copy + recip
                    # (DVE/ACT work that never blocks the PE queue)
                    q0 = qc * QC
                    n_kt = q0 // 128 + 4
                    pr = b * 2 + h
                    qT_h = qkvT["qT"][b][h * DH:(h + 1) * DH, :]
                    kT_h = qkvT["kT"][b][h * DH:(h + 1) * DH, :]
                    ps_ctx = psC_pool.tile([128, QC], F32, name="ps_ctx",
                                           tag="ps_ctx")
                    for kg in range(n_kt // 2):
                        ka, kb = 2 * kg, 2 * kg + 1
                        offa = max(0, ka * 128 - q0)
                        offb = max(0, kb * 128 - q0)
                        ps_s = psS_pool.tile([128, 1024], F32, name="ps_s",
                                             tag="ps_s")
                        # kb's block is packed at column 512 (width 512-offb)
                        # so the exp range [offa:1024-offb] has no dead gap
                        nc.tensor.matmul(
                            ps_s[:, offa:512],
                            kT_h[:, ka * 128:(ka + 1) * 128],
                            qT_h[:, q0 + offa:q0 + 512],
                            start=True, stop=True,
                        )
                        nc.tensor.matmul(
                            ps_s[:, 512:1024 - offb],
                            kT_h[:, kb * 128:(kb + 1) * 128],
                            qT_h[:, q0 + offb:q0 + 512],
                            start=True, stop=True,
                        )
                        ex = exp_pool.tile([128, 1024], BF16, name="ex", tag="ex")
                        nc.scalar.activation(
                            out=ex[:, offa:1024 - offb], in_=ps_s[:, offa:1024 - offb],
                            func=mybir.ActivationFunctionType.Exp,
                            scale=INV_SCALE,
                        )
                        # causal mask: multiplicative 0/1 on the diagonal
                        # tiles, applied AFTER exp (off the PE->ACT path)
                        if ka * 128 >= q0:
                            nc.vector.tensor_mul(
                                out=ex[:, offa:offa + 128],
                                in0=ex[:, offa:offa + 128],
                                in1=tri01[:],
                            )
                        if kb * 128 >= q0:
                            nc.vector.tensor_mul(
                                out=ex[:, 512:640],
                                in0=ex[:, 512:640],
                                in1=tri01[:],
                            )
                        nc.tensor.matmul(
                            ps_ctx[0:DH + 1, offa:512],
                            vaug[:, pr, ka, :],
                            ex[:, offa:512],
                            start=(ka == 0), stop=False,
                            skip_group_check=True,
                        )
                        nc.tensor.matmul(
                            ps_ctx[0:DH + 1, offb:512],
                            vaug[:, pr, kb, :],
                            ex[:, 512:1024 - offb],
                            start=False, stop=(kb == n_kt - 1),
                            skip_group_check=True,
                        )
                    nc.vector.tensor_copy(
                        ctxu[0:DH + 1, pr, q0:q0 + 512],
                        ps_ctx[0:DH + 1, :],
                    )
                    # r = 1/l = exp(-ln(l)); ln/exp share the attention exp's
                    # ACT table (reciprocal would force table reloads), and
                    # the exp writes f32r (verifier-approved rounding op)
                    ln_f = rpool.tile([65, QC], F32, name="ln_f", tag="ln_f")
                    nc.scalar.activation(
                        out=ln_f[64:65, :], in_=ctxu[64:65, pr, q0:q0 + QC],
                        func=mybir.ActivationFunctionType.Ln)
                    r_t = rpool.tile([65, QC], F32R, name="r_t", tag="r_t")
                    nc.scalar.activation(
                        out=r_t[64:65, :], in_=ln_f[64:65, :],
                        func=mybir.ActivationFunctionType.Exp, scale=-1.0)
                    r_tiles[(b, qc, h)] = r_t

                def emit_bcnorm(b, qc, h):
                    # deferred: broadcast r along 64 partitions (PE outer
                    # product) + normalize ctx into the chunk's bf16 tile
                    q0 = qc * QC
                    pr = b * 2 + h
                    if h == 0:
                        cn_tiles[(b, qc)] = cn_pool.tile(
                            [128, QC], BF16, name="cn", tag="cn")
                    cn = cn_tiles[(b, qc)]
                    r_t = r_tiles.pop((b, qc, h))
                    bc = psBC_pool.tile([128, QC], F32, name="bc", tag="bc")
                    nc.tensor.matmul(
                        bc[0:DH, :],
                        ones_r[64:65, 0:DH],
                        r_t[64:65, :],
                        start=True, stop=True,
                    )
                    nc.vector.tensor_mul(
                        out=cn[h * DH:(h + 1) * DH, :],
                        in0=ctxu[0:DH, pr, q0:q0 + QC],
                        in1=bc[0:DH, :],
                    )
                    if h == 1:
                        # chunk complete: ship to the A2A input buffer
                        # (token groups 2qc, 2qc+1). Sync queue — the gpsimd
                        # queue blocks on in-flight collectives.
                        nc.sync.dma_start(
                            out=a2a_in[b][:].rearrange("g p n -> p g n")[
                                :, 2 * qc:2 * qc + 2, :],
                            in_=cn[:].rearrange("p (g n) -> p g n", g=2),
                        )
                        del cn_tiles[(b, qc)]

                def emit_a2a(b):
                    nc.gpsimd.collective_compute(
                        "AllToAll",
                        mybir.AluOpType.bypass,
                        ins=[a2a_in[b][:]],
                        outs=[a2a_out[b][:]],
                        replica_groups=[list(range(NC))],
                    )

                ctxf_tiles = {}

                def emit_ctxf_load(b):
                    ctxf = cf_pool.tile([128, KT_D, TPC], BF16, name="ctxf",
                                        tag="ctxf")
                    nc.gpsimd.dma_start(
                        out=ctxf[:],
                        in_=a2a_out[b][:].rearrange("kt p n -> p kt n"))
                    ctxf_tiles[b] = ctxf

                def emit_outproj(b):
                    # token-sharded output projection for this core's TPC
                    # tokens of batch b, using the full Wo
                    if b not in ctxf_tiles:
                        emit_ctxf_load(b)
                    ctxf = ctxf_tiles.pop(b)
                    for og in range(KT_D // 2):
                        # two out-dim tiles share one PSUM bank
                        ps_o = psO_pool.tile([128, 2, TPC], F32, name="ps_o",
                                             tag="ps_o")
                        for sub in range(2):
                            ot = og * 2 + sub
                            for kt in range(KT_D):
                                nc.tensor.matmul(
                                    ps_o[:, sub, :],
                                    wo_sb[:, kt, ot * 128:(ot + 1) * 128],
                                    ctxf[:, kt, :],
                                    start=(kt == 0), stop=(kt == KT_D - 1),
                                    skip_group_check=True,
                                )
                        yo = yo_pool.tile([128, 2, TPC], F32, name="yo", tag="yo")
                        for sub in range(2):
                            ot = og * 2 + sub
                            nc.vector.tensor_scalar_add(
                                out=yo[:, sub, :], in0=ps_o[:, sub, :],
                                scalar1=bo_sb[:, ot:ot + 1],
                            )
                        nc.sync.dma_start(
                            out=yT2[og * 256:(og + 1) * 256,
                                    b * TPC:(b + 1) * TPC].rearrange(
                                "(ot p) n -> p ot n", p=128),
                            in_=yo[:],
                        )

                # pipeline: bc/normalize of (qc, h) is emitted one h-slot
                # late so the PE queue never waits on the reciprocal chain;
                # batch b's A2A+outproj hide under batch b+1's attention.
                pending = []
                for b in range(B):
                    for qc in range(NQC):
                        emit_D(b, qc, 0)
                        if pending:
                            pending.pop(0)()
                        emit_D(b, qc, 1)
                        emit_bcnorm(b, qc, 0)
                        pending.append(
                            (lambda bb=b, qq=qc: (
                                emit_bcnorm(bb, qq, 1),
                                emit_a2a(bb) if qq == NQC - 1 else None)))
                        if b == 1 and qc == 1:
                            # b0's gathered-ctx load, well after its A2A and
                            # before b1's A2A occupies the gpsimd queue
                            pending.append(lambda: emit_ctxf_load(0))
                for fn in pending:
                    fn()
                # batch 0's out-proj here: its PE work fills the bubble while
                # batch 1's A2A is in flight; batch 1's follows
                emit_outproj(0)
                emit_outproj(1)

    _split_waits(nc)
    return nc


def _r(ap):
    return ap.bitcast(F32R)


def _tile_w(w):
    # [D, N] -> [128, KT_D * N]: contraction tile kt on partitions
    w = np.asarray(w)
    n = w.shape[1]
    return np.ascontiguousarray(
        w.reshape(KT_D, 128, n).transpose(1, 0, 2).reshape(128, KT_D * n))


def kernel(x, mask, Wq, bq, Wk, bk, Wv, bv, Wo, bo, trace=False):
    import ml_dtypes
    bf16 = ml_dtypes.bfloat16
    x = np.asarray(x, dtype=np.float32).reshape(T, D)
    xT = np.ascontiguousarray(x.T).astype(bf16)
    Wo_bf = _tile_w(np.asarray(Wo, np.float32)).astype(bf16)
    bo_f = np.ascontiguousarray(
        np.asarray(bo, np.float32).reshape(KT_D, 128).T)
    in_maps = []
    for c in range(NC):
        sl = slice(c * HG, (c + 1) * HG)
        in_maps.append({
            "xT": xT,
            "wq": _tile_w(np.asarray(Wq, np.float32)[:, sl]).astype(bf16),
            "wk": _tile_w(np.asarray(Wk, np.float32)[:, sl]).astype(bf16),
            "wv": _tile_w(np.asarray(Wv, np.float32)[:, sl]).astype(bf16),
            "wo": Wo_bf,
            "bq": np.ascontiguousarray(np.asarray(bq, np.float32)[sl].reshape(HG, 1)),
            "bk": np.ascontiguousarray(np.asarray(bk, np.float32)[sl].reshape(HG, 1)),
            "bv": np.ascontiguousarray(np.asarray(bv, np.float32)[sl].reshape(HG, 1)),
            "bo": bo_f,
        })
    nc = build_module()
    res = run_bass_kernel_spmd(nc, in_maps, core_ids=list(range(NC)), trace=trace)
    out = np.empty((B, S, D), dtype=np.float32)
    for c in range(NC):
        y = res.results[c]["yT2"]  # [D, B*TPC]
        for b in range(B):
            out[b, c * TPC:(c + 1) * TPC, :] = y[:, b * TPC:(b + 1) * TPC].T
    if trace:
        kernel.last_results = res
    return out.reshape(B, S, D)


# revision 24
# speedup vs baseline: 1.8672x; 1.1034x over previous
"""Multi-head causal self-attention (B=2, S=2048, D=1024, H=16) on 8 TRN2 cores.

Sharding: head-parallel for QKV+attention (core c owns heads {2c, 2c+1}),
token-parallel for the output projection (core c owns tokens
[256c, 256c+256) of each batch), bridged by a per-batch AllToAll of the
normalized context — 8x less wire traffic than gathering or
reduce-scattering partial outputs, since nothing is replicated.

The PE clock gate (HAM) re-throttles to 1.2GHz whenever any 3.4us window
has an idle gap; the attention kg chain (PE scores -> ACT exp -> DVE mask
-> PE ctx) alone always has such gaps. So the emission engine interleaves
dependency-free "filler" matmuls into the PE queue: batch 1's QKV
projection + V-transposes fill batch 0's attention; batch 0's output
projection fills batch 1's attention. bc/normalize consumers are emitted
one h-slot late so the PE queue never waits on the l-reciprocal chain.

Per core (matmul operands bf16; PSUM accumulation f32):
  stage B: Q^T/K^T/V^T = (x @ W{q,k,v}[:, c-slice] + b)^T   [128, 4096]
  stage C: V^T -> V_aug [tok, 65] tiles (col 64 = ones -> l row)
  stage D: scores^T = K^T.T @ Q^T tiles (PE), exp (ACT, scale=1/8,
           diagonal blocks packed so no masked column is exp'd),
           post-exp multiplicative causal mask (DVE), ctx^T accum (PE)
  stage E: l row -> r = exp(-ln(l)) (ACT, same table as exp), PE
           outer-product broadcast, normalize -> bf16
  stage F: per batch: AllToAll ctx chunks, then out = Wo^T-tiles @
           ctx_full + bo for this core's tokens, full Wo

Host: x pre-transposed bf16; weights host-pre-tiled to [p, kt, n] so all
loads are contiguous; output reassembled token-wise from yT2.
"""

import sys

for p in ("/opt/trn_rl_repo", "/root/.axon_site/_ro/trn_rl_repo"):
    if p not in sys.path:
        sys.path.insert(0, p)

from collections import deque

import numpy as np

import bass_rust
import concourse.bass as bass
import concourse.mybir as mybir
from concourse.bass_utils import run_bass_kernel_spmd
from concourse.masks import make_identity
from concourse.tile import TileContext

B, S, D = 2, 2048, 1024
H, DH = 16, 64
T = B * S              # 4096 tokens
NC = 8                 # cores
HG = D // NC           # 128 qkv dims per core (2 heads)
KT_D = D // 128        # 8 contraction tiles over d_model
QC = 512               # q-chunk width
NQC = S // QC          # 4 q-chunks per batch
TPC = S // NC          # 256 tokens per core per batch (out-proj sharding)
INV_SCALE = 1.0 / float(np.sqrt(DH))  # 1/8
F32 = mybir.dt.float32
F32R = mybir.dt.float32r
BF16 = mybir.dt.bfloat16


def _split_waits(nc, max_waits=1):
    """This walrus build accepts one sync-wait per instruction; Tile sometimes
    emits more. Split extras into preceding NoOps on the same engine."""
    n = 0
    for f in nc.m.functions:
        for bb in f.blocks:
            out = []
            for inst in bb.instructions:
                si = getattr(inst, "sync_info", None)
                if si is not None and si.on_wait and len(si.on_wait) > max_waits:
                    waits = list(si.on_wait)
                    head, rest = waits[:-max_waits], waits[-max_waits:]
                    k = 0
                    while head:
                        chunk, head = head[:max_waits], head[max_waits:]
                        out.append(mybir.InstNoOp(
                            name=f"{inst.name}-wsplit-{k}", ins=[], outs=[],
                            engine=inst.engine,
                            sync_info=bass_rust.SyncInfo(on_wait=chunk, on_update=[]),
                        ))
                        k += 1
                    si.on_wait = rest
                    n += 1
                out.append(inst)
            bb.instructions = out
    return n


def build_module():
    nc = bass.Bass()

    # weights arrive host-pre-tiled ([p, kt, n] flattened) so the loads are
    # fully contiguous DMAs instead of 256B-descriptor gather patterns
    xT = nc.dram_tensor("xT", [D, T], BF16, kind="ExternalInput")
    wq = nc.dram_tensor("wq", [128, KT_D * HG], BF16, kind="ExternalInput")
    wk = nc.dram_tensor("wk", [128, KT_D * HG], BF16, kind="ExternalInput")
    wv = nc.dram_tensor("wv", [128, KT_D * HG], BF16, kind="ExternalInput")
    wo = nc.dram_tensor("wo", [128, KT_D * D], BF16, kind="ExternalInput")
    bq = nc.dram_tensor("bq", [HG, 1], F32, kind="ExternalInput")
    bk = nc.dram_tensor("bk", [HG, 1], F32, kind="ExternalInput")
    bv = nc.dram_tensor("bv", [HG, 1], F32, kind="ExternalInput")
    bo = nc.dram_tensor("bo", [128, KT_D], F32, kind="ExternalInput")
    # output: this core's TPC tokens of each batch, all D dims
    yT2 = nc.dram_tensor("yT2", [D, B * TPC], F32, kind="ExternalOutput")

    # AllToAll buffers: [token-group/src-rank, 128, TPC]
    a2a_in = [nc.dram_tensor(f"a2i{b}", [NC, HG, TPC], BF16) for b in range(B)]
    a2a_out = [nc.dram_tensor(f"a2o{b}", [NC, HG, TPC], BF16) for b in range(B)]

    with TileContext(nc) as tc:
        with tc.tile_pool(name="persist", bufs=1) as pp:
            w_sb = {}
            for name, dram in (("wq", wq), ("wk", wk), ("wv", wv)):
                t = pp.tile([128, KT_D, HG], BF16, name=f"{name}_sb", tag=f"{name}_sb")
                nc.sync.dma_start(out=t[:], in_=dram[:].rearrange("p (kt n) -> p kt n", n=HG))
                w_sb[name] = t
            # gpsimd queue: the 2MB Wo load must not head-of-line block the
            # first xt loads on the sync queue
            wo_sb = pp.tile([128, KT_D, D], BF16, name="wo_sb", tag="wo_sb")
            nc.gpsimd.dma_start(out=wo_sb[:], in_=wo[:].rearrange("p (kt n) -> p kt n", n=D))
            b_sb = {}
            for name, dram in (("bq", bq), ("bk", bk), ("bv", bv)):
                t = pp.tile([HG, 1], F32, name=f"{name}_sb", tag=f"{name}_sb")
                nc.sync.dma_start(out=t[:], in_=dram[:])
                b_sb[name] = t
            bo_sb = pp.tile([128, KT_D], F32, name="bo_sb", tag="bo_sb")
            nc.gpsimd.dma_start(out=bo_sb[:], in_=bo[:])

            ident_f = pp.tile([128, 128], F32, name="ident_f", tag="ident_f")
            make_identity(nc, ident_f[:])
            ident = pp.tile([128, 128], BF16, name="ident", tag="ident")
            nc.vector.tensor_copy(ident[:], ident_f[:])
            # multiplicative causal mask for a diagonal 128x128 tile of
            # scores^T: keep [r, c] where r <= c (k <= q)
            tri_f = pp.tile([128, 128], F32, name="tri_f", tag="tri_f")
            nc.gpsimd.memset(tri_f[:], 1.0)
            nc.gpsimd.affine_select(
                out=tri_f[:], in_=tri_f[:],
                compare_op=mybir.AluOpType.is_ge, fill=0.0,
                base=0, pattern=[[1, 128]], channel_multiplier=-1,
            )
            tri01 = pp.tile([128, 128], BF16, name="tri01", tag="tri01")
            nc.vector.tensor_copy(tri01[:], tri_f[:])
            # ones row at partition 64 (base partition of the l row)
            ones_f = pp.tile([65, DH], F32, name="ones_f", tag="ones_f")
            nc.vector.memset(ones_f[:], 1.0)
            ones_r = pp.tile([65, DH], F32R, name="ones_r", tag="ones_r")
            nc.vector.tensor_copy(ones_r[:], ones_f[:])
            ones128 = pp.tile([128, B * 2 * (S // 128)], F32, name="ones128",
                              tag="ones128")
            nc.vector.memset(ones128[:], 1.0)

            qkvT = {}
            for name in ("qT", "kT", "vT"):
                qkvT[name] = [pp.tile([128, S], BF16, name=f"{name}{b}", tag=f"{name}{b}")
                              for b in range(B)]

            vaug = pp.tile([128, B * 2, S // 128, DH + 1], BF16, name="vaug", tag="vaug")
            nc.vector.tensor_copy(vaug[:, :, :, DH:DH + 1], ones128[:, :])
            # [65 used partitions, pair, q]; row 64 = l
            ctxu = pp.tile([128, B * 2, S], F32, name="ctxu", tag="ctxu")

            def emit_C(b, h, g, pst):
                # V^T -> V_aug transposes for 8 ktiles; pst: [128, >=512] bf16
                # PSUM region
                pr = b * 2 + h
                for j in range(8):
                    kt = g * 8 + j
                    nc.tensor.transpose(
                        out=pst[:, j * DH:(j + 1) * DH],
                        in_=qkvT["vT"][b][h * DH:(h + 1) * DH,
                                          kt * 128:(kt + 1) * 128],
                        identity=ident[h * DH:(h + 1) * DH,
                                       h * DH:(h + 1) * DH],
                    )
                nc.vector.tensor_copy(
                    vaug[:, pr, g * 8:(g + 1) * 8, 0:DH],
                    pst[:, 0:512],
                )

            # ---------------- stage B+C for batch 0 ----------------
            with (
                tc.tile_pool(name="xt0_pool", bufs=3) as xt0_pool,
                tc.tile_pool(name="psB", bufs=1, space="PSUM") as psB_pool,
                tc.tile_pool(name="psT", bufs=2, space="PSUM") as psT_pool,
            ):
                for tq in range(2):
                    t0 = tq * 1024
                    ps = [psB_pool.tile([128, 512], F32, name=f"psB{i}",
                                        tag=f"psB{i}") for i in range(6)]
                    for kt in range(KT_D):
                        xt = xt0_pool.tile([128, 1024], BF16, name="xt", tag="xt")
                        nc.sync.dma_start(
                            out=xt[:],
                            in_=xT[kt * 128:(kt + 1) * 128, t0:t0 + 1024])
                        for pi, wname in enumerate(("wq", "wk", "wv")):
                            for nch in range(2):
                                nc.tensor.matmul(
                                    ps[pi * 2 + nch][:],
                                    w_sb[wname][:, kt, :],
                                    xt[:, nch * 512:(nch + 1) * 512],
                                    start=(kt == 0), stop=(kt == KT_D - 1),
                                )
                    for pi, (dname, bname) in enumerate(
                            (("qT", "bq"), ("kT", "bk"), ("vT", "bv"))):
                        for nch in range(2):
                            nc.vector.tensor_scalar_add(
                                out=qkvT[dname][0][:, t0 + nch * 512:
                                                   t0 + (nch + 1) * 512],
                                in0=ps[pi * 2 + nch][:],
                                scalar1=b_sb[bname][:, 0:1],
                            )
                for h in range(2):
                    for g in range(2):
                        pst = psT_pool.tile([128, 512], BF16, name="pst", tag="pst")
                        emit_C(0, h, g, pst[:])

            # ------- stages D-F + interleaved B(b1)/C(b1)/F'(b0) -------
            with (
                tc.tile_pool(name="psS", bufs=2, space="PSUM") as psS_pool,
                tc.tile_pool(name="psC", bufs=1, space="PSUM") as psC_pool,
                tc.tile_pool(name="mps", bufs=1, space="PSUM") as mps_pool,
                tc.tile_pool(name="psBI", bufs=1, space="PSUM") as psBI_pool,
                tc.tile_pool(name="xt_pool", bufs=16) as xt_pool,
                tc.tile_pool(name="exp_pool", bufs=4) as exp_pool,
                tc.tile_pool(name="rpool", bufs=4) as rpool,
                tc.tile_pool(name="cn_pool", bufs=4) as cn_pool,
                tc.tile_pool(name="cf_pool", bufs=2) as cf_pool,
                tc.tile_pool(name="yo_pool", bufs=2) as yo_pool,
            ):
                r_tiles = {}
                cn_tiles = {}
                ctxf_tiles = {}
                filler = deque()   # dependency-free PE work units
                pending = []       # deferred bc/normalize slots

                # ---- B(b1) interleave units ----
                # 2-chunk groups share 8 live [128,1024] xt tiles; each
                # 512-chunk runs 3 passes (q, k, v) over 2 accumulator banks.
                xt1_tiles = {}     # tqgroup -> list of 8 tiles

                def u_load(tqg, half):
                    def f():
                        tiles = xt1_tiles.setdefault(tqg, [])
                        for kt in range(half * 4, half * 4 + 4):
                            xt = xt_pool.tile([128, 1024], BF16, name="xt1",
                                              tag="xt1")
                            nc.sync.dma_start(
                                out=xt[:],
                                in_=xT[kt * 128:(kt + 1) * 128,
                                       S + tqg * 1024: S + tqg * 1024 + 1024])
                            tiles.append(xt)
                    return f

                def u_pass(tqg, ch, wname, bname, acc_tag):
                    # one full projection pass for 512-token chunk ch of
                    # group tqg: 8 accumulate MMs + bias drain
                    def f():
                        acc = psBI_pool.tile([128, 512], F32, name=acc_tag,
                                             tag=acc_tag)
                        tiles = xt1_tiles[tqg]
                        for kt in range(KT_D):
                            nc.tensor.matmul(
                                acc[:],
                                w_sb[wname][:, kt, :],
                                tiles[kt][:, ch * 512:(ch + 1) * 512],
                                start=(kt == 0), stop=(kt == KT_D - 1),
                            )
                        t0 = tqg * 1024 + ch * 512
                        nc.vector.tensor_scalar_add(
                            out=qkvT[wname.replace("w", "") + "T"][1][:, t0:t0 + 512],
                            in0=acc[:],
                            scalar1=b_sb[bname][:, 0:1],
                        )
                    return f

                def u_c1(h, g):
                    def f():
                        # transpose scratch borrowed from a psS tile (bitcast
                        # f32 bank region to bf16)
                        ps_t = psS_pool.tile([128, 1024], F32, name="ps_s",
                                             tag="ps_s")
                        emit_C(1, h, g, ps_t[:].bitcast(BF16))
                    return f

                for tqg in range(2):
                    filler.append(u_load(tqg, 0))
                    filler.append(u_load(tqg, 1))
                    for ch in range(2):
                        for wname, bname, acc in (("wq", "bq", "accA"),
                                                  ("wk", "bk", "accB"),
                                                  ("wv", "bv", "accA")):
                            filler.append(u_pass(tqg, ch, wname, bname, acc))
                    if tqg == 1:
                        for h in range(2):
                            for g in range(2):
                                filler.append(u_c1(h, g))

                def emit_D(b, qc, h, fill_every=2):
                    q0 = qc * QC
                    n_kt = q0 // 128 + 4
                    pr = b * 2 + h
                    qT_h = qkvT["qT"][b][h * DH:(h + 1) * DH, :]
                    kT_h = qkvT["kT"][b][h * DH:(h + 1) * DH, :]
                    ps_ctx = psC_pool.tile([128, QC], F32, name="ps_ctx",
                                           tag="ps_ctx")
                    for kg in range(n_kt // 2):
                        ka, kb = 2 * kg, 2 * kg + 1
                        offa = max(0, ka * 128 - q0)
                        offb = max(0, kb * 128 - q0)
                        ps_s = psS_pool.tile([128, 1024], F32, name="ps_s",
                                             tag="ps_s")
                        # kb's block packed at column 512 (width 512-offb):
                        # the exp range [offa:1024-offb] has no dead gap
                        nc.tensor.matmul(
                            ps_s[:, offa:512],
                            kT_h[:, ka * 128:(ka + 1) * 128],
                            qT_h[:, q0 + offa:q0 + 512],
                            start=True, stop=True,
                        )
                        nc.tensor.matmul(
                            ps_s[:, 512:1024 - offb],
                            kT_h[:, kb * 128:(kb + 1) * 128],
                            qT_h[:, q0 + offb:q0 + 512],
                            start=True, stop=True,
                        )
                        ex = exp_pool.tile([128, 1024], BF16, name="ex", tag="ex")
                        nc.scalar.activation(
                            out=ex[:, offa:1024 - offb],
                            in_=ps_s[:, offa:1024 - offb],
                            func=mybir.ActivationFunctionType.Exp,
                            scale=INV_SCALE,
                        )
                        if ka * 128 >= q0:
                            nc.vector.tensor_mul(
                                out=ex[:, offa:offa + 128],
                                in0=ex[:, offa:offa + 128],
                                in1=tri01[:],
                            )
                        if kb * 128 >= q0:
                            nc.vector.tensor_mul(
                                out=ex[:, 512:640],
                                in0=ex[:, 512:640],
                                in1=tri01[:],
                            )
                        nc.tensor.matmul(
                            ps_ctx[0:DH + 1, offa:512],
                            vaug[:, pr, ka, :],
                            ex[:, offa:512],
                            start=(ka == 0), stop=False,
                            skip_group_check=True,
                        )
                        nc.tensor.matmul(
                            ps_ctx[0:DH + 1, offb:512],
                            vaug[:, pr, kb, :],
                            ex[:, 512:1024 - offb],
                            start=False, stop=(kb == n_kt - 1),
                            skip_group_check=True,
                        )
                        if filler and kg % fill_every == fill_every - 1:
                            filler.popleft()()
                    nc.vector.tensor_copy(
                        ctxu[0:DH + 1, pr, q0:q0 + 512],
                        ps_ctx[0:DH + 1, :],
                    )
                    # r = 1/l = exp(-ln(l)): ln/exp share the attention exp's
                    # ACT table; the exp writes f32r (a rounding op, so the
                    # f32r matmul consumer passes BIR verification)
                    ln_f = rpool.tile([65, QC], F32, name="ln_f", tag="ln_f")
                    nc.scalar.activation(
                        out=ln_f[64:65, :], in_=ctxu[64:65, pr, q0:q0 + QC],
                        func=mybir.ActivationFunctionType.Ln)
                    r_t = rpool.tile([65, QC], F32R, name="r_t", tag="r_t")
                    nc.scalar.activation(
                        out=r_t[64:65, :], in_=ln_f[64:65, :],
                        func=mybir.ActivationFunctionType.Exp, scale=-1.0)
                    r_tiles[(b, qc, h)] = r_t

                def emit_bcnorm(b, qc, h):
                    # deferred one h-slot: the PE bc matmul's reciprocal input
                    # is long since ready, so the PE queue never stalls here
                    q0 = qc * QC
                    pr = b * 2 + h
                    if h == 0:
                        cn_tiles[(b, qc)] = cn_pool.tile(
                            [128, QC], BF16, name="cn", tag="cn")
                    cn = cn_tiles[(b, qc)]
                    r_t = r_tiles.pop((b, qc, h))
                    bc = mps_pool.tile([128, QC], F32, name="bc", tag="mps")
                    nc.tensor.matmul(
                        bc[0:DH, :],
                        ones_r[64:65, 0:DH],
                        r_t[64:65, :],
                        start=True, stop=True,
                    )
                    nc.vector.tensor_mul(
                        out=cn[h * DH:(h + 1) * DH, :],
                        in0=ctxu[0:DH, pr, q0:q0 + QC],
                        in1=bc[0:DH, :],
                    )
                    if h == 1:
                        # chunk complete: ship to the A2A input buffer
                        # (token groups 2qc, 2qc+1). Sync queue — the gpsimd
                        # queue blocks on in-flight collectives.
                        nc.sync.dma_start(
                            out=a2a_in[b][:].rearrange("g p n -> p g n")[
                                :, 2 * qc:2 * qc + 2, :],
                            in_=cn[:].rearrange("p (g n) -> p g n", g=2),
                        )
                        del cn_tiles[(b, qc)]

                def emit_a2a(b):
                    nc.gpsimd.collective_compute(
                        "AllToAll",
                        mybir.AluOpType.bypass,
                        ins=[a2a_in[b][:]],
                        outs=[a2a_out[b][:]],
                        replica_groups=[list(range(NC))],
                    )

                def emit_ctxf_load(b):
                    ctxf = cf_pool.tile([128, KT_D, TPC], BF16, name="ctxf",
                                        tag="ctxf")
                    nc.gpsimd.dma_start(
                        out=ctxf[:],
                        in_=a2a_out[b][:].rearrange("kt p n -> p kt n"))
                    ctxf_tiles[b] = ctxf

                def u_outproj(b, og):
                    # two out-dim tiles of batch b's token-sharded output
                    # projection (shares the mps bank with bc)
                    def f():
                        ctxf = ctxf_tiles[b]
                        ps_o = mps_pool.tile([128, 2, TPC], F32, name="ps_o",
                                             tag="mps")
                        for sub in range(2):
                            ot = og * 2 + sub
                            for kt in range(KT_D):
                                nc.tensor.matmul(
                                    ps_o[:, sub, :],
                                    wo_sb[:, kt, ot * 128:(ot + 1) * 128],
                                    ctxf[:, kt, :],
                                    start=(kt == 0), stop=(kt == KT_D - 1),
                                    skip_group_check=True,
                                )
                        yo = yo_pool.tile([128, 2, TPC], F32, name="yo", tag="yo")
                        for sub in range(2):
                            ot = og * 2 + sub
                            nc.vector.tensor_scalar_add(
                                out=yo[:, sub, :], in0=ps_o[:, sub, :],
                                scalar1=bo_sb[:, ot:ot + 1],
                            )
                        nc.sync.dma_start(
                            out=yT2[og * 256:(og + 1) * 256,
                                    b * TPC:(b + 1) * TPC].rearrange(
                                "(ot p) n -> p ot n", p=128),
                            in_=yo[:],
                        )
                    return f

                for b in range(B):
                    for qc in range(NQC):
                        # batch 1's attention interleaves the (sparser)
                        # out-proj units less often
                        fe = 2 if b == 0 else 5
                        emit_D(b, qc, 0, fill_every=fe)
                        if pending:
                            pending.pop(0)()
                        emit_D(b, qc, 1, fill_every=fe)
                        emit_bcnorm(b, qc, 0)
                        pending.append(
                            (lambda bb=b, qq=qc: (
                                emit_bcnorm(bb, qq, 1),
                                emit_a2a(bb) if qq == NQC - 1 else None)))
                    if b == 0:
                        # drain any leftover B(b1)/C(b1) units before D(b1)
                        # needs their outputs
                        while filler:
                            filler.popleft()()
                        # b0's bc/norm tail + A2A, then queue its out-proj
                        # units as D(b1) filler
                        while pending:
                            pending.pop(0)()
                        emit_ctxf_load(0)
                        for og in range(KT_D // 2):
                            filler.append(u_outproj(0, og))
                while pending:
                    pending.pop(0)()
                while filler:
                    filler.popleft()()
                emit_ctxf_load(1)
                for og in range(KT_D // 2):
                    u_outproj(1, og)()

    _split_waits(nc)
    return nc


def _tile_w(w):
    # [D, N] -> [128, KT_D * N]: contraction tile kt on partitions
    w = np.asarray(w)
    n = w.shape[1]
    return np.ascontiguousarray(
        w.reshape(KT_D, 128, n).transpose(1, 0, 2).reshape(128, KT_D * n))


def kernel(x, mask, Wq, bq, Wk, bk, Wv, bv, Wo, bo, trace=False):
    import ml_dtypes
    bf16 = ml_dtypes.bfloat16
    x = np.asarray(x, dtype=np.float32).reshape(T, D)
    xT = np.ascontiguousarray(x.T).astype(bf16)
    Wo_bf = _tile_w(np.asarray(Wo, np.float32)).astype(bf16)
    bo_f = np.ascontiguousarray(
        np.asarray(bo, np.float32).reshape(KT_D, 128).T)
    in_maps = []
    for c in range(NC):
        sl = slice(c * HG, (c + 1) * HG)
        in_maps.append({
            "xT": xT,
            "wq": _tile_w(np.asarray(Wq, np.float32)[:, sl]).astype(bf16),
            "wk": _tile_w(np.asarray(Wk, np.float32)[:, sl]).astype(bf16),
            "wv": _tile_w(np.asarray(Wv, np.float32)[:, sl]).astype(bf16),
            "wo": Wo_bf,
            "bq": np.ascontiguousarray(np.asarray(bq, np.float32)[sl].reshape(HG, 1)),
            "bk": np.ascontiguousarray(np.asarray(bk, np.float32)[sl].reshape(HG, 1)),
            "bv": np.ascontiguousarray(np.asarray(bv, np.float32)[sl].reshape(HG, 1)),
            "bo": bo_f,
        })
    nc = build_module()
    res = run_bass_kernel_spmd(nc, in_maps, core_ids=list(range(NC)), trace=trace)
    out = np.empty((B, S, D), dtype=np.float32)
    for c in range(NC):
        y = res.results[c]["yT2"]  # [D, B*TPC]
        for b in range(B):
            out[b, c * TPC:(c + 1) * TPC, :] = y[:, b * TPC:(b + 1) * TPC].T
    if trace:
        kernel.last_results = res
    return out.reshape(B, S, D)
